# revision 36
# baseline (speedup 1.0000x reference)
"""Bidirectional cross-attention kernel for Trainium2, SPMD over 8 NeuronCores.

Reference (per batch b, heads K=8, head dim D=32, N=128*128 pixels):
    q   = softmax_d(Wq @ x)
    for branch j in {1,2}:
        key   = softmax_n(Wk_j @ ref_j)          # softmax over the pixel dim
        v     = Wv_j @ ref_j
        ctx_j = key @ v^T                        # [K,D,D]
        out_j = per-pixel  q @ ctx_j^T
    y = Wo @ concat(out_1, out_2)

Sharding: 8 cores = batch(4) x head-group(2).  Each core owns 4 of the 8
heads for its batch: projections, softmaxes, ctx and the out einsum are
fully head-local; the final Wo projection is computed as a partial sum
over the core's 256 (of 512) concat channels, and the host adds the two
partial outputs per batch.  No cross-core communication on device.

Key algebraic restructure vs the straightforward version: since the
per-pixel out einsum and Wo are both linear, fold them:
    y = sum_j Wo_j @ (ctx_norm_j^T @ q) = (sum_j Wo_j @ ctx_norm_j) @ q
      = WF @ q
where WF is a per-head-block [256, 32] matrix built once from the tiny
ctx blocks.  This turns the whole output phase into a single
[256x128] @ [128xN] matmul stream and removes the concat buffer and its
per-pixel zk normalization entirely (zk folds into WF).

zk (sum over pixels of exp(k)) is obtained for free by appending a
ones-column to the streamed v operand of the ctx matmul: ctx is computed
as ek^T-stationary x [v | 1], so column D of the ctx PSUM block is zk,
already transposed onto partitions.

Numerics: bf16 matmul inputs (host-cast), fp32 PSUM accumulation, fp32
scalar/vector math.  Softmaxes skip max-subtraction (logits ~N(0,1), exp
is safe in fp32).

SBUF layout: tensors with >128 channels are stored as [128, k*cols] with
128-channel k-tiles side by side in the free dim.  Key/value tensors are
kept in transposed [pixel, channel] layout (needed for the ctx einsum,
whose contraction runs over pixels); v tiles are 129 wide (128 channels
+ a ones column).
"""

import numpy as np
import ml_dtypes

import concourse.bass as bass
import concourse.bacc as bacc
import concourse.tile as tile
from concourse import mybir
from concourse.bass_utils import run_bass_kernel_spmd

BF16 = mybir.dt.bfloat16
F32 = mybir.dt.float32
AF = mybir.ActivationFunctionType

B, C, H, W = 4, 256, 128, 128
K, D = 8, 32
N = H * W
N_CORES = 8


def build_nc(n_loc=N):
    nc = bacc.Bacc("TRN2", target_bir_lowering=False, debug=False,
                   num_devices=N_CORES)

    nt = n_loc // 128        # 128-pixel tiles (128)
    nt512 = n_loc // 512     # 512-pixel tiles (32)

    # ---- I/O (weights pre-transposed, head-group-sliced, k-tiled on host) --
    # x/r/y are chunk-pair-major [npair*128, 2048]: pair pr rows hold pixels
    # 1024pr..1024pr+1024, col = 1024*(ch%2) + 512k + c.  Each paired DMA is
    # one fully contiguous [128, 2048] block -> 4 KiB packets per partition
    # row (the DMA engines' per-packet overhead dominates below ~4 KiB).
    npair = n_loc // 1024
    x = nc.declare_dram_parameter("x", [npair * 128, 2048], BF16,
                                  isOutput=False)
    r1 = nc.declare_dram_parameter("r1", [npair * 128, 2048], BF16,
                                   isOutput=False)
    r2 = nc.declare_dram_parameter("r2", [npair * 128, 2048], BF16,
                                   isOutput=False)
    # wq: [128, 2*128]  col chunk 128k = Wq.T[128k:128k+128, our 128 channels]
    wq = nc.declare_dram_parameter("wq", [128, 2 * 128], BF16, isOutput=False)
    # wkv_j: [128, 2*256] col chunk 256k = [WkT | WvT](our heads)[128k:, :]
    wkv1 = nc.declare_dram_parameter("wkv1", [128, 2 * 256], BF16, isOutput=False)
    wkv2 = nc.declare_dram_parameter("wkv2", [128, 2 * 256], BF16, isOutput=False)
    # wo: [128, 2*256]  col chunk 256j = Wo.T[(branch j, our heads) rows, :]
    wo = nc.declare_dram_parameter("wo", [128, 2 * 256], BF16, isOutput=False)
    ones4 = nc.declare_dram_parameter("ones4", [128, 32], BF16, isOutput=False)
    ones4T = nc.declare_dram_parameter("ones4T", [128, 128], BF16, isOutput=False)

    y = nc.declare_dram_parameter("y", [npair * 128, 2048], BF16,
                                  isOutput=True)

    refs = [r1, r2]

    with tile.TileContext(nc) as tc:
        with (
            tc.tile_pool(name="weights", bufs=1) as wpool,
            tc.tile_pool(name="persist", bufs=1) as ppool,
            tc.tile_pool(name="io", bufs=6) as iopool,
            tc.tile_pool(name="work", bufs=3) as wkpool,
        ):
            # ---- weights / constants.  Only wkv1 + ones4 are loaded up
            # front; the rest are issued between the first chunk DMAs so the
            # input stream starts as early as possible.
            wkv_t = [wpool.tile([128, 2 * 256], BF16, tag=f"wkv{j}",
                                name=f"wkv_t{j}") for j in range(2)]
            nc.gpsimd.dma_start(wkv_t[0][:], wkv1[:, :])
            ones4_t = wpool.tile([128, 32], BF16, tag="o4")
            nc.sync.dma_start(ones4_t[:], ones4[:, :])
            wq_t = wpool.tile([128, 2 * 128], BF16, tag="wq")
            ones4T_t = wpool.tile([128, 128], BF16, tag="o4T")
            wo_t = wpool.tile([128, 2 * 256], BF16, tag="wo")

            expq = ppool.tile([128, n_loc], BF16, tag="expq")
            nzc = (nt512 + 3) // 4
            zqr = ppool.tile([128, 512 * nzc], BF16, tag="zqr")
            recips = ppool.tile([128, 2], F32, tag="recips")
            compact = ppool.tile([128, 64], BF16, tag="compact")
            wft_sb = ppool.tile([128, 256], BF16, tag="wft_sb")

            CH = 4               # kv: 128-pixel tiles per chunk
            nch = nt // CH       # 32 chunks per branch

            with (
                tc.tile_pool(name="kvstage", bufs=1) as kvpool,
                tc.tile_pool(name="psA", bufs=2, space="PSUM") as psA,
                tc.tile_pool(name="psAcc", bufs=1, space="PSUM") as psAcc,
                tc.tile_pool(name="psQ", bufs=2, space="PSUM") as psQ,
            ):
                ekt_all = kvpool.tile([128, nt * 128], BF16, tag="ekt_all")
                vt_all = kvpool.tile([128, nt * 129], BF16, tag="vt_all")
                vt_v = vt_all.rearrange("p (t c) -> p t c", c=129)
                # ones column per v tile (survives both branches: the per-
                # chunk v copies only touch cols 0:128 of each 129-block)
                nc.vector.memset(vt_v[:, :, 128:129], 1.0)
                # ctx blob: branch j at cols 129j..129j+129; col 129j+128 = zk
                ctx_ps = psAcc.tile([128, 2 * 129], F32, tag="ctx")

                r_hold = [None]

                def pass1(j, ch):
                    if ch % 2 == 0:
                        pr = ch // 2
                        r_hold[0] = iopool.tile(
                            [128, 2048], BF16, tag="rchunk",
                            name=f"r_{j}_{pr}")
                        src = refs[j][128 * pr:128 * (pr + 1), :]
                        if j == 0 and ch == 0:
                            # first pair: 4 quarter DMAs on both queues so
                            # the very first kv matmuls start sooner
                            for qi in range(4):
                                eng = nc.sync if qi % 2 == 0 else nc.gpsimd
                                eng.dma_start(
                                    r_hold[0][:, 512 * qi:512 * (qi + 1)],
                                    src[:, 512 * qi:512 * (qi + 1)])
                        else:
                            dma_eng = nc.sync if pr % 2 == 0 else nc.gpsimd
                            dma_eng.dma_start(r_hold[0][:], src)
                    r_t = r_hold[0]
                    off = 1024 * (ch % 2)
                    kv_ps = psA.tile([128, CH * 256], F32, tag="kv",
                                     name=f"kv_{j}_{ch}")
                    for t in range(CH):
                        for k in range(2):
                            nc.tensor.matmul(
                                kv_ps[:, 256 * t:256 * (t + 1)],
                                r_t[:, off + 512 * k + 128 * t:
                                       off + 512 * k + 128 * (t + 1)],
                                wkv_t[j][:, 256 * k:256 * (k + 1)],
                                start=(k == 0), stop=(k == 1),
                            )
                    ek_sl = ekt_all[:, ch * CH * 128:(ch + 1) * CH * 128]
                    nc.scalar.activation(
                        ek_sl.rearrange("p (t c) -> p t c", t=CH),
                        kv_ps[:].rearrange("p (t c) -> p t c", t=CH)[:, :, 0:128],
                        AF.Exp,
                    )
                    nc.vector.tensor_copy(
                        vt_v[:, ch * CH:(ch + 1) * CH, 0:128],
                        kv_ps[:].rearrange("p (t c) -> p t c", t=CH)[:, :, 128:256],
                    )

                def pass2(j, ch):
                    # ctx accumulation: ek-tile stationary, [v | 1] streamed.
                    # out[c, d] = sum_pix ek[pix, c] v[pix, d]; col 128 = zk.
                    for t in range(ch * CH, (ch + 1) * CH):
                        nc.tensor.matmul(
                            ctx_ps[:, 129 * j:129 * (j + 1)],
                            ekt_all[:, 128 * t:128 * (t + 1)],
                            vt_all[:, 129 * t:129 * (t + 1)],
                            start=(t == 0), stop=(t == nt - 1),
                        )

                x_hold = [None]

                def qchunk(i):
                    base = i * 512
                    if i % 2 == 0:
                        pr = i // 2
                        x_hold[0] = iopool.tile([128, 2048], BF16,
                                                tag="xchunk", name=f"x_{pr}")
                        dma_eng = nc.gpsimd if pr % 2 == 0 else nc.sync
                        dma_eng.dma_start(
                            x_hold[0][:], x[128 * pr:128 * (pr + 1), :])
                    x_t = x_hold[0]
                    off = 1024 * (i % 2)
                    q_ps = psQ.tile([128, 512], F32, tag="q", name=f"q_{i}")
                    for k in range(2):
                        nc.tensor.matmul(
                            q_ps[:], wq_t[:, 128 * k:128 * (k + 1)],
                            x_t[:, off + 512 * k:off + 512 * (k + 1)],
                            start=(k == 0), stop=(k == 1),
                        )
                    nc.scalar.activation(
                        expq[:, base:base + 512], q_ps[:], AF.Exp)

                def zqgroup(tc4):
                    # zq = per-head sums of expq (4 col-tiled concurrent MMs),
                    # reciprocal, then matmul-broadcast back over the 32
                    # partitions of each head and normalize expq in place.
                    zq_ps = psQ.tile([128, 512], F32, tag="q", name=f"zq_{tc4}")
                    for u in range(4):
                        t = 4 * tc4 + u
                        nc.tensor.matmul(
                            zq_ps[32 * u:32 * u + 32, :], ones4_t[:],
                            expq[:, 512 * t:512 * (t + 1)],
                            start=True, stop=True,
                            tile_position=(0, 32 * u),
                        )
                    zq_f = wkpool.tile([128, 512], F32, tag="zq_f",
                                       name=f"zqf_{tc4}")
                    nc.vector.reciprocal_approx_fast(zq_f[:], zq_ps[:])
                    nc.vector.tensor_copy(
                        zqr[:, 512 * tc4:512 * (tc4 + 1)], zq_f[:])
                    for t in range(4 * tc4, 4 * tc4 + 4):
                        u = t % 4
                        zqb_ps = psQ.tile([128, 512], F32, tag="q",
                                          name=f"zqb_{t}")
                        nc.tensor.matmul(
                            zqb_ps[:], ones4T_t[32 * u:32 * u + 4, :],
                            zqr[32 * u:32 * u + 4,
                                512 * tc4:512 * (tc4 + 1)],
                            start=True, stop=True,
                            tile_position=(32 * u, 0),
                        )
                        nc.vector.tensor_mul(
                            expq[:, 512 * t:512 * (t + 1)],
                            expq[:, 512 * t:512 * (t + 1)],
                            zqb_ps[:],
                        )

                # ---- branches; q projection spread across both to even out
                # the DMA load (r + x/2 per branch).  Remaining weight DMAs
                # are dripped in between the chunk DMAs. ----
                for j in range(2):
                    for ch in range(nch):
                        pass1(j, ch)
                        if j == 0 and ch == 0:
                            nc.gpsimd.dma_start(wq_t[:], wq[:, :])
                        if j == 0 and ch == 3:
                            nc.sync.dma_start(ones4T_t[:], ones4T[:, :])
                        if j == 0 and ch == 6:
                            nc.sync.dma_start(wkv_t[1][:], wkv2[:, :])
                        if j == 1 and ch == 2:
                            nc.gpsimd.dma_start(wo_t[:], wo[:, :])
                        if ch > 0:
                            pass2(j, ch - 1)
                        if ch % 2 == 1:
                            qchunk(16 * j + ch // 2)
                        if ch % 8 == 7:
                            zqgroup(4 * j + ch // 8)
                    pass2(j, nch - 1)

                # ---- WF = sum_j WoT_j^T-blocks @ ctx_norm_j, per head -----
                for j in range(2):
                    nc.vector.reciprocal_approx_fast(
                        recips[:, j:j + 1],
                        ctx_ps[:, 129 * j + 128:129 * j + 129])
                for j in range(2):
                    for h in range(4):
                        nc.vector.tensor_scalar_mul(
                            compact[32 * h:32 * (h + 1), 32 * j:32 * j + 32],
                            ctx_ps[32 * h:32 * (h + 1),
                                   129 * j + 32 * h:129 * j + 32 * (h + 1)],
                            recips[32 * h:32 * (h + 1), j:j + 1],
                        )
                wft_ps = psA.tile([128, 256], F32, tag="wft", bufs=1)
                for h in range(4):
                    for j in range(2):
                        nc.tensor.matmul(
                            wft_ps[32 * h:32 * (h + 1), :],
                            compact[32 * h:32 * (h + 1), 32 * j:32 * j + 32],
                            wo_t[32 * h:32 * (h + 1), 256 * j:256 * (j + 1)],
                            start=(j == 0), stop=(j == 1),
                            tile_position=(32 * h, 32 * h),
                        )
                nc.vector.tensor_copy(wft_sb[:], wft_ps[:])

            # ======= Phase C: y = WF @ expq, streamed over pixel tiles ======
            with (
                tc.tile_pool(name="psC", bufs=3, space="PSUM") as psC,
                tc.tile_pool(name="ysb", bufs=5) as ysbpool,
            ):
                y_hold = [None]
                for t in range(nt512):
                    y_ps = psC.tile([128, 1024], F32, tag="y", name=f"y_{t}")
                    for m in range(2):
                        nc.tensor.matmul(
                            y_ps[:, 512 * m:512 * (m + 1)],
                            wft_sb[:, 128 * m:128 * (m + 1)],
                            expq[:, 512 * t:512 * (t + 1)],
                            start=True, stop=True,
                        )
                    if t % 2 == 0:
                        y_hold[0] = ysbpool.tile([128, 2048], BF16, tag="ysb",
                                                 name=f"ysb_{t // 2}")
                    y_sb = y_hold[0]
                    sl = y_sb[:, 1024 * (t % 2):1024 * (t % 2) + 1024]
                    if t % 2 == 0:
                        nc.vector.tensor_copy(sl, y_ps[:])
                    else:
                        nc.scalar.copy(sl, y_ps[:])
                        pr = t // 2
                        dma_eng = nc.sync if pr % 2 == 0 else nc.gpsimd
                        dma_eng.dma_start(
                            y[128 * pr:128 * (pr + 1), :], y_sb[:])

    nc.compile()
    return nc


def _consts():
    ones4 = np.zeros((128, 32), dtype=ml_dtypes.bfloat16)
    for col in range(32):
        a = col % 4
        ones4[32 * a:32 * (a + 1), col] = 1
    ones4T = np.zeros((128, 128), dtype=ml_dtypes.bfloat16)
    for u in range(4):
        for a in range(4):
            ones4T[32 * u + a, 32 * a:32 * (a + 1)] = 1
    return ones4, ones4T


def _ktile(wT):
    """[C_in, C_out] -> [128, (C_in//128)*C_out] k-tiles along the free dim."""
    kin = wT.shape[0] // 128
    return np.concatenate([wT[128 * k:128 * (k + 1), :] for k in range(kin)], axis=1)


def _chunkmajor(arr, n_loc=N):
    """[256, n] -> [npair*128, 2048]: pair pr holds pixels 1024pr..1024pr+1024
    at col = 1024*(ch%2) + 512k + c (k = channel k-tile)."""
    npair = n_loc // 1024
    # [k, p, pr, e, c] -> [pr, p, e, k, c]
    a = arr.reshape(2, 128, npair, 2, 512).transpose(2, 1, 3, 0, 4)
    return a.reshape(npair * 128, 2048)


def _unchunk_y(yarr, n_loc=N):
    """[npair*128, 2048] -> [256, n]: per pair pr, col 1024e + 512m + c of
    partition p is y[128m + p, 512*(2pr + e) + c]."""
    npair = n_loc // 1024
    # [pr, p, e, m, c] -> [m, p, pr, e, c]
    a = yarr.reshape(npair, 128, 2, 2, 512).transpose(3, 1, 0, 2, 4)
    return a.reshape(C, n_loc)


def make_in_maps(x, ref_1, ref_2, Wq, Wk1, Wk2, Wv1, Wv2, Wo, n_loc=N):
    bf = ml_dtypes.bfloat16
    ones4, ones4T = _consts()
    xf = np.asarray(x).reshape(B, C, -1)
    r1f = np.asarray(ref_1).reshape(B, C, -1)
    r2f = np.asarray(ref_2).reshape(B, C, -1)
    WqT, WoT = np.asarray(Wq).T, np.asarray(Wo).T
    WkT = [np.asarray(Wk1).T, np.asarray(Wk2).T]
    WvT = [np.asarray(Wv1).T, np.asarray(Wv2).T]
    gw = {}
    for g in range(2):
        sl = slice(128 * g, 128 * (g + 1))
        wq_g = np.ascontiguousarray(_ktile(WqT[:, sl])).astype(bf)
        wkv_g = [np.ascontiguousarray(
            _ktile(np.concatenate([WkT[j][:, sl], WvT[j][:, sl]], axis=1))
        ).astype(bf) for j in range(2)]
        # Wo rows for our concat channels: branch j block at cols 256j
        wo_g = np.ascontiguousarray(np.concatenate(
            [WoT[128 * g:128 * (g + 1), :],
             WoT[256 + 128 * g:256 + 128 * (g + 1), :]],
            axis=1)).astype(bf)
        gw[g] = (wq_g, wkv_g[0], wkv_g[1], wo_g)
    in_maps = []
    for core in range(N_CORES):
        b, g = core // 2, core % 2
        wq_g, wkv1_g, wkv2_g, wo_g = gw[g]
        in_maps.append({
            "x": np.ascontiguousarray(_chunkmajor(xf[b, :, :n_loc], n_loc)).astype(bf),
            "r1": np.ascontiguousarray(_chunkmajor(r1f[b, :, :n_loc], n_loc)).astype(bf),
            "r2": np.ascontiguousarray(_chunkmajor(r2f[b, :, :n_loc], n_loc)).astype(bf),
            "wq": wq_g, "wkv1": wkv1_g, "wkv2": wkv2_g, "wo": wo_g,
            "ones4": ones4, "ones4T": ones4T,
        })
    return in_maps


_NC_CACHE = {}


def kernel(x, ref_1, ref_2, Wq, Wk1, Wk2, Wv1, Wv2, Wo, _trace=False):
    n_loc = N
    if n_loc not in _NC_CACHE:
        _NC_CACHE[n_loc] = build_nc(n_loc)
    nc = _NC_CACHE[n_loc]
    in_maps = make_in_maps(x, ref_1, ref_2, Wq, Wk1, Wk2, Wv1, Wv2, Wo, n_loc)
    res = run_bass_kernel_spmd(nc, in_maps, core_ids=list(range(N_CORES)),
                               trace=_trace)
    out = np.empty((B, C, n_loc), dtype=np.float32)
    for b in range(B):
        out[b] = (_unchunk_y(res.results[2 * b]["y"].astype(np.float32), n_loc)
                  + _unchunk_y(res.results[2 * b + 1]["y"].astype(np.float32), n_loc))
    if _trace:
        kernel.last_results = res
    return out.reshape(B, C, H, W)


# revision 38
# speedup vs baseline: 1.1491x; 1.1491x over previous
"""Bidirectional cross-attention kernel for Trainium2, SPMD over 8 NeuronCores.

Reference (per batch b, heads K=8, head dim D=32, N=128*128 pixels):
    q   = softmax_d(Wq @ x)
    for branch j in {1,2}:
        key   = softmax_n(Wk_j @ ref_j)          # softmax over the pixel dim
        v     = Wv_j @ ref_j
        ctx_j = key @ v^T                        # [K,D,D]
        out_j = per-pixel  q @ ctx_j^T
    y = Wo @ concat(out_1, out_2)

Sharding: 8 cores = batch(4) x head-group(2).  Each core owns 4 of the 8
heads for its batch: projections, softmaxes, ctx and the out einsum are
fully head-local; the final Wo projection is computed as a partial sum
over the core's 256 (of 512) concat channels, and the host adds the two
partial outputs per batch.  No cross-core communication on device.

Key algebraic restructure vs the straightforward version: since the
per-pixel out einsum and Wo are both linear, fold them:
    y = sum_j Wo_j @ (ctx_norm_j^T @ q) = (sum_j Wo_j @ ctx_norm_j) @ q
      = WF @ q
where WF is a per-head-block [256, 32] matrix built once from the tiny
ctx blocks.  This turns the whole output phase into a single
[256x128] @ [128xN] matmul stream and removes the concat buffer and its
per-pixel zk normalization entirely (zk folds into WF).

zk (sum over pixels of exp(k)) is obtained for free by appending a
ones-column to the streamed v operand of the ctx matmul: ctx is computed
as ek^T-stationary x [v | 1], so column D of the ctx PSUM block is zk,
already transposed onto partitions.

Numerics: bf16 matmul inputs (host-cast), fp32 PSUM accumulation, fp32
scalar/vector math.  Softmaxes skip max-subtraction (logits ~N(0,1), exp
is safe in fp32).

SBUF layout: tensors with >128 channels are stored as [128, k*cols] with
128-channel k-tiles side by side in the free dim.  Key/value tensors are
kept in transposed [pixel, channel] layout (needed for the ctx einsum,
whose contraction runs over pixels); v tiles are 129 wide (128 channels
+ a ones column).
"""

import numpy as np
import ml_dtypes

import concourse.bass as bass
import concourse.bacc as bacc
import concourse.tile as tile
from concourse import mybir
from concourse.bass_utils import run_bass_kernel_spmd

BF16 = mybir.dt.bfloat16
F32 = mybir.dt.float32
AF = mybir.ActivationFunctionType

B, C, H, W = 4, 256, 128, 128
K, D = 8, 32
N = H * W
N_CORES = 8


def build_nc(n_loc=N):
    nc = bacc.Bacc("TRN2", target_bir_lowering=False, debug=False,
                   num_devices=N_CORES)

    nt = n_loc // 128        # 128-pixel tiles (128)
    nt512 = n_loc // 512     # 512-pixel tiles (32)

    # ---- I/O (weights pre-transposed, head-group-sliced, k-tiled on host) --
    # x/r/y are chunk-pair-major [npair*128, 2048]: pair pr rows hold pixels
    # 1024pr..1024pr+1024, col = 1024*(ch%2) + 512k + c.  Each paired DMA is
    # one fully contiguous [128, 2048] block -> 4 KiB packets per partition
    # row (the DMA engines' per-packet overhead dominates below ~4 KiB).
    npair = n_loc // 1024
    x = nc.declare_dram_parameter("x", [npair * 128, 2048], BF16,
                                  isOutput=False)
    r1 = nc.declare_dram_parameter("r1", [npair * 128, 2048], BF16,
                                   isOutput=False)
    r2 = nc.declare_dram_parameter("r2", [npair * 128, 2048], BF16,
                                   isOutput=False)
    # wq: [128, 2*128]  col chunk 128k = Wq.T[128k:128k+128, our 128 channels]
    wq = nc.declare_dram_parameter("wq", [128, 2 * 128], BF16, isOutput=False)
    # wkv_j: [128, 2*256] col chunk 256k = [WkT | WvT](our heads)[128k:, :]
    wkv1 = nc.declare_dram_parameter("wkv1", [128, 2 * 256], BF16, isOutput=False)
    wkv2 = nc.declare_dram_parameter("wkv2", [128, 2 * 256], BF16, isOutput=False)
    # wo: [128, 2*256]  col chunk 256j = Wo.T[(branch j, our heads) rows, :]
    wo = nc.declare_dram_parameter("wo", [128, 2 * 256], BF16, isOutput=False)
    ones4 = nc.declare_dram_parameter("ones4", [128, 32], BF16, isOutput=False)
    ones4T = nc.declare_dram_parameter("ones4T", [128, 128], BF16, isOutput=False)

    y = nc.declare_dram_parameter("y", [npair * 128, 2048], BF16,
                                  isOutput=True)

    refs = [r1, r2]

    with tile.TileContext(nc) as tc:
        with (
            tc.tile_pool(name="weights", bufs=1) as wpool,
            tc.tile_pool(name="persist", bufs=1) as ppool,
            tc.tile_pool(name="io", bufs=6) as iopool,
            tc.tile_pool(name="work", bufs=3) as wkpool,
        ):
            # ---- weights / constants.  Only wkv1 + ones4 are loaded up
            # front; the rest are issued between the first chunk DMAs so the
            # input stream starts as early as possible.
            wkv_t = [wpool.tile([128, 2 * 256], BF16, tag=f"wkv{j}",
                                name=f"wkv_t{j}") for j in range(2)]
            nc.gpsimd.dma_start(wkv_t[0][:], wkv1[:, :])
            ones4_t = wpool.tile([128, 32], BF16, tag="o4")
            nc.sync.dma_start(ones4_t[:], ones4[:, :])
            wq_t = wpool.tile([128, 2 * 128], BF16, tag="wq")
            ones4T_t = wpool.tile([128, 128], BF16, tag="o4T")
            wo_t = wpool.tile([128, 2 * 256], BF16, tag="wo")

            expq = ppool.tile([128, n_loc], BF16, tag="expq")
            nzc = (nt512 + 3) // 4
            zqr = ppool.tile([128, 512 * nzc], BF16, tag="zqr")
            recips = ppool.tile([128, 2], F32, tag="recips")
            compact = ppool.tile([128, 64], BF16, tag="compact")
            wft_sb = ppool.tile([128, 256], BF16, tag="wft_sb")

            CH = 4               # kv: 128-pixel tiles per chunk
            nch = nt // CH       # 32 chunks per branch

            with (
                tc.tile_pool(name="kvstage", bufs=1) as kvpool,
                tc.tile_pool(name="psA", bufs=2, space="PSUM") as psA,
                tc.tile_pool(name="psAcc", bufs=1, space="PSUM") as psAcc,
                tc.tile_pool(name="psQ", bufs=2, space="PSUM") as psQ,
            ):
                ekt_all = kvpool.tile([128, nt * 128], BF16, tag="ekt_all")
                vt_all = kvpool.tile([128, nt * 129], BF16, tag="vt_all")
                vt_v = vt_all.rearrange("p (t c) -> p t c", c=129)
                # ones column per v tile (survives both branches: the per-
                # chunk v copies only touch cols 0:128 of each 129-block)
                nc.vector.memset(vt_v[:, :, 128:129], 1.0)
                # ctx blob: branch j at cols 129j..129j+129; col 129j+128 = zk
                ctx_ps = psAcc.tile([128, 2 * 129], F32, tag="ctx")

                r_hold = [None]

                def pass1(j, ch):
                    if ch % 2 == 0:
                        pr = ch // 2
                        r_hold[0] = iopool.tile(
                            [128, 2048], BF16, tag="rchunk",
                            name=f"r_{j}_{pr}")
                        src = refs[j][128 * pr:128 * (pr + 1), :]
                        if j == 0 and ch == 0:
                            # first pair: 4 quarter DMAs on both queues so
                            # the very first kv matmuls start sooner
                            for qi in range(4):
                                eng = nc.sync if qi % 2 == 0 else nc.gpsimd
                                eng.dma_start(
                                    r_hold[0][:, 512 * qi:512 * (qi + 1)],
                                    src[:, 512 * qi:512 * (qi + 1)])
                        else:
                            dma_eng = nc.sync if pr % 2 == 0 else nc.gpsimd
                            dma_eng.dma_start(r_hold[0][:], src)
                    r_t = r_hold[0]
                    off = 1024 * (ch % 2)
                    kv_ps = psA.tile([128, CH * 256], F32, tag="kv",
                                     name=f"kv_{j}_{ch}")
                    for t in range(CH):
                        for k in range(2):
                            nc.tensor.matmul(
                                kv_ps[:, 256 * t:256 * (t + 1)],
                                r_t[:, off + 512 * k + 128 * t:
                                       off + 512 * k + 128 * (t + 1)],
                                wkv_t[j][:, 256 * k:256 * (k + 1)],
                                start=(k == 0), stop=(k == 1),
                            )
                    ek_sl = ekt_all[:, ch * CH * 128:(ch + 1) * CH * 128]
                    nc.scalar.activation(
                        ek_sl.rearrange("p (t c) -> p t c", t=CH),
                        kv_ps[:].rearrange("p (t c) -> p t c", t=CH)[:, :, 0:128],
                        AF.Exp,
                    )
                    nc.vector.tensor_copy(
                        vt_v[:, ch * CH:(ch + 1) * CH, 0:128],
                        kv_ps[:].rearrange("p (t c) -> p t c", t=CH)[:, :, 128:256],
                    )

                def pass2(j, ch):
                    # ctx accumulation: ek-tile stationary, [v | 1] streamed.
                    # out[c, d] = sum_pix ek[pix, c] v[pix, d]; col 128 = zk.
                    for t in range(ch * CH, (ch + 1) * CH):
                        nc.tensor.matmul(
                            ctx_ps[:, 129 * j:129 * (j + 1)],
                            ekt_all[:, 128 * t:128 * (t + 1)],
                            vt_all[:, 129 * t:129 * (t + 1)],
                            start=(t == 0), stop=(t == nt - 1),
                        )

                x_hold = [None]

                def qchunk(i):
                    base = i * 512
                    if i % 2 == 0:
                        pr = i // 2
                        x_hold[0] = iopool.tile([128, 2048], BF16,
                                                tag="xchunk", name=f"x_{pr}")
                        dma_eng = nc.gpsimd if pr % 2 == 0 else nc.sync
                        dma_eng.dma_start(
                            x_hold[0][:], x[128 * pr:128 * (pr + 1), :])
                    x_t = x_hold[0]
                    off = 1024 * (i % 2)
                    q_ps = psQ.tile([128, 512], F32, tag="q", name=f"q_{i}")
                    for k in range(2):
                        nc.tensor.matmul(
                            q_ps[:], wq_t[:, 128 * k:128 * (k + 1)],
                            x_t[:, off + 512 * k:off + 512 * (k + 1)],
                            start=(k == 0), stop=(k == 1),
                        )
                    nc.scalar.activation(
                        expq[:, base:base + 512], q_ps[:], AF.Exp)

                def zqgroup(tc4):
                    # zq = per-head sums of expq (4 col-tiled concurrent MMs),
                    # reciprocal, then matmul-broadcast back over the 32
                    # partitions of each head and normalize expq in place.
                    zq_ps = psQ.tile([128, 512], F32, tag="q", name=f"zq_{tc4}")
                    for u in range(4):
                        t = 4 * tc4 + u
                        nc.tensor.matmul(
                            zq_ps[32 * u:32 * u + 32, :], ones4_t[:],
                            expq[:, 512 * t:512 * (t + 1)],
                            start=True, stop=True,
                            tile_position=(0, 32 * u),
                        )
                    zq_f = wkpool.tile([128, 512], F32, tag="zq_f",
                                       name=f"zqf_{tc4}")
                    nc.vector.reciprocal_approx_fast(zq_f[:], zq_ps[:])
                    nc.vector.tensor_copy(
                        zqr[:, 512 * tc4:512 * (tc4 + 1)], zq_f[:])
                    for t in range(4 * tc4, 4 * tc4 + 4):
                        u = t % 4
                        zqb_ps = psQ.tile([128, 512], F32, tag="q",
                                          name=f"zqb_{t}")
                        nc.tensor.matmul(
                            zqb_ps[:], ones4T_t[32 * u:32 * u + 4, :],
                            zqr[32 * u:32 * u + 4,
                                512 * tc4:512 * (tc4 + 1)],
                            start=True, stop=True,
                            tile_position=(32 * u, 0),
                        )
                        nc.vector.tensor_mul(
                            expq[:, 512 * t:512 * (t + 1)],
                            expq[:, 512 * t:512 * (t + 1)],
                            zqb_ps[:],
                        )

                # ---- branches; q projection spread across both to even out
                # the DMA load (r + x/2 per branch).  Remaining weight DMAs
                # are dripped in between the chunk DMAs. ----
                # qchunks: 8 spread over ch 1..15, 8 packed into ch 16..23,
                # so the last zqgroup (and with it the whole q-normalize
                # tail) finishes ~5 chunks before the branch ends instead of
                # sitting serially on the critical path into phase C.
                ZQ_AT = {10: 0, 17: 1, 21: 2, 26: 3}
                for j in range(2):
                    for ch in range(nch):
                        pass1(j, ch)
                        if j == 0 and ch == 1:
                            nc.gpsimd.dma_start(wq_t[:], wq[:, :])
                        if j == 0 and ch == 5:
                            nc.sync.dma_start(ones4T_t[:], ones4T[:, :])
                        if j == 0 and ch == 8:
                            nc.sync.dma_start(wkv_t[1][:], wkv2[:, :])
                        if j == 1 and ch == 2:
                            nc.gpsimd.dma_start(wo_t[:], wo[:, :])
                        if ch > 0:
                            pass2(j, ch - 1)
                        if ch % 2 == 1 and ch <= 15:
                            qchunk(16 * j + (ch - 1) // 2)
                        elif 16 <= ch <= 23:
                            qchunk(16 * j + 8 + (ch - 16))
                        if ch in ZQ_AT:
                            zqgroup(4 * j + ZQ_AT[ch])
                    pass2(j, nch - 1)

                # ---- WF = sum_j WoT_j^T-blocks @ ctx_norm_j, per head -----
                for j in range(2):
                    nc.vector.reciprocal_approx_fast(
                        recips[:, j:j + 1],
                        ctx_ps[:, 129 * j + 128:129 * j + 129])
                for j in range(2):
                    for h in range(4):
                        nc.vector.tensor_scalar_mul(
                            compact[32 * h:32 * (h + 1), 32 * j:32 * j + 32],
                            ctx_ps[32 * h:32 * (h + 1),
                                   129 * j + 32 * h:129 * j + 32 * (h + 1)],
                            recips[32 * h:32 * (h + 1), j:j + 1],
                        )
                wft_ps = psA.tile([128, 256], F32, tag="wft", bufs=1)
                for h in range(4):
                    for j in range(2):
                        nc.tensor.matmul(
                            wft_ps[32 * h:32 * (h + 1), :],
                            compact[32 * h:32 * (h + 1), 32 * j:32 * j + 32],
                            wo_t[32 * h:32 * (h + 1), 256 * j:256 * (j + 1)],
                            start=(j == 0), stop=(j == 1),
                            tile_position=(32 * h, 32 * h),
                        )
                nc.vector.tensor_copy(wft_sb[:], wft_ps[:])

            # ======= Phase C: y = WF @ expq, streamed over pixel tiles ======
            with (
                tc.tile_pool(name="psC", bufs=3, space="PSUM") as psC,
                tc.tile_pool(name="ysb", bufs=5) as ysbpool,
            ):
                y_hold = [None]
                for t in range(nt512):
                    y_ps = psC.tile([128, 1024], F32, tag="y", name=f"y_{t}")
                    for m in range(2):
                        nc.tensor.matmul(
                            y_ps[:, 512 * m:512 * (m + 1)],
                            wft_sb[:, 128 * m:128 * (m + 1)],
                            expq[:, 512 * t:512 * (t + 1)],
                            start=True, stop=True,
                        )
                    if t % 2 == 0:
                        y_hold[0] = ysbpool.tile([128, 2048], BF16, tag="ysb",
                                                 name=f"ysb_{t // 2}")
                    y_sb = y_hold[0]
                    sl = y_sb[:, 1024 * (t % 2):1024 * (t % 2) + 1024]
                    if t % 2 == 0:
                        nc.vector.tensor_copy(sl, y_ps[:])
                        if t == 0:
                            # kick the output stream one cast earlier: the
                            # out-DMA drain sets phase C's end time
                            nc.sync.dma_start(y[0:128, 0:1024],
                                              y_sb[:, 0:1024])
                    else:
                        nc.scalar.copy(sl, y_ps[:])
                        pr = t // 2
                        if t == 1:
                            nc.gpsimd.dma_start(y[0:128, 1024:2048],
                                                y_sb[:, 1024:2048])
                        else:
                            dma_eng = nc.sync if pr % 2 == 0 else nc.gpsimd
                            dma_eng.dma_start(
                                y[128 * pr:128 * (pr + 1), :], y_sb[:])

    nc.compile()
    return nc


def _consts():
    ones4 = np.zeros((128, 32), dtype=ml_dtypes.bfloat16)
    for col in range(32):
        a = col % 4
        ones4[32 * a:32 * (a + 1), col] = 1
    ones4T = np.zeros((128, 128), dtype=ml_dtypes.bfloat16)
    for u in range(4):
        for a in range(4):
            ones4T[32 * u + a, 32 * a:32 * (a + 1)] = 1
    return ones4, ones4T


def _ktile(wT):
    """[C_in, C_out] -> [128, (C_in//128)*C_out] k-tiles along the free dim."""
    kin = wT.shape[0] // 128
    return np.concatenate([wT[128 * k:128 * (k + 1), :] for k in range(kin)], axis=1)


def _chunkmajor(arr, n_loc=N):
    """[256, n] -> [npair*128, 2048]: pair pr holds pixels 1024pr..1024pr+1024
    at col = 1024*(ch%2) + 512k + c (k = channel k-tile)."""
    npair = n_loc // 1024
    # [k, p, pr, e, c] -> [pr, p, e, k, c]
    a = arr.reshape(2, 128, npair, 2, 512).transpose(2, 1, 3, 0, 4)
    return a.reshape(npair * 128, 2048)


def _unchunk_y(yarr, n_loc=N):
    """[npair*128, 2048] -> [256, n]: per pair pr, col 1024e + 512m + c of
    partition p is y[128m + p, 512*(2pr + e) + c]."""
    npair = n_loc // 1024
    # [pr, p, e, m, c] -> [m, p, pr, e, c]
    a = yarr.reshape(npair, 128, 2, 2, 512).transpose(3, 1, 0, 2, 4)
    return a.reshape(C, n_loc)


def make_in_maps(x, ref_1, ref_2, Wq, Wk1, Wk2, Wv1, Wv2, Wo, n_loc=N):
    bf = ml_dtypes.bfloat16
    ones4, ones4T = _consts()
    xf = np.asarray(x).reshape(B, C, -1)
    r1f = np.asarray(ref_1).reshape(B, C, -1)
    r2f = np.asarray(ref_2).reshape(B, C, -1)
    WqT, WoT = np.asarray(Wq).T, np.asarray(Wo).T
    WkT = [np.asarray(Wk1).T, np.asarray(Wk2).T]
    WvT = [np.asarray(Wv1).T, np.asarray(Wv2).T]
    gw = {}
    for g in range(2):
        sl = slice(128 * g, 128 * (g + 1))
        wq_g = np.ascontiguousarray(_ktile(WqT[:, sl])).astype(bf)
        wkv_g = [np.ascontiguousarray(
            _ktile(np.concatenate([WkT[j][:, sl], WvT[j][:, sl]], axis=1))
        ).astype(bf) for j in range(2)]
        # Wo rows for our concat channels: branch j block at cols 256j
        wo_g = np.ascontiguousarray(np.concatenate(
            [WoT[128 * g:128 * (g + 1), :],
             WoT[256 + 128 * g:256 + 128 * (g + 1), :]],
            axis=1)).astype(bf)
        gw[g] = (wq_g, wkv_g[0], wkv_g[1], wo_g)
    in_maps = []
    for core in range(N_CORES):
        b, g = core // 2, core % 2
        wq_g, wkv1_g, wkv2_g, wo_g = gw[g]
        in_maps.append({
            "x": np.ascontiguousarray(_chunkmajor(xf[b, :, :n_loc], n_loc)).astype(bf),
            "r1": np.ascontiguousarray(_chunkmajor(r1f[b, :, :n_loc], n_loc)).astype(bf),
            "r2": np.ascontiguousarray(_chunkmajor(r2f[b, :, :n_loc], n_loc)).astype(bf),
            "wq": wq_g, "wkv1": wkv1_g, "wkv2": wkv2_g, "wo": wo_g,
            "ones4": ones4, "ones4T": ones4T,
        })
    return in_maps


_NC_CACHE = {}


def kernel(x, ref_1, ref_2, Wq, Wk1, Wk2, Wv1, Wv2, Wo, _trace=False):
    n_loc = N
    if n_loc not in _NC_CACHE:
        _NC_CACHE[n_loc] = build_nc(n_loc)
    nc = _NC_CACHE[n_loc]
    in_maps = make_in_maps(x, ref_1, ref_2, Wq, Wk1, Wk2, Wv1, Wv2, Wo, n_loc)
    res = run_bass_kernel_spmd(nc, in_maps, core_ids=list(range(N_CORES)),
                               trace=_trace)
    out = np.empty((B, C, n_loc), dtype=np.float32)
    for b in range(B):
        out[b] = (_unchunk_y(res.results[2 * b]["y"].astype(np.float32), n_loc)
                  + _unchunk_y(res.results[2 * b + 1]["y"].astype(np.float32), n_loc))
    if _trace:
        kernel.last_results = res
    return out.reshape(B, C, H, W)


# revision 48
# speedup vs baseline: 1.1523x; 1.0028x over previous
"""Bidirectional cross-attention kernel for Trainium2, SPMD over 8 NeuronCores.

Reference (per batch b, heads K=8, head dim D=32, N=128*128 pixels):
    q   = softmax_d(Wq @ x)
    for branch j in {1,2}:
        key   = softmax_n(Wk_j @ ref_j)          # softmax over the pixel dim
        v     = Wv_j @ ref_j
        ctx_j = key @ v^T                        # [K,D,D]
        out_j = per-pixel  q @ ctx_j^T
    y = Wo @ concat(out_1, out_2)

Sharding: 8 cores = batch(4) x head-group(2).  Each core owns 4 of the 8
heads for its batch: projections, softmaxes, ctx and the out einsum are
fully head-local; the final Wo projection is computed as a partial sum
over the core's 256 (of 512) concat channels, and the host adds the two
partial outputs per batch.  No cross-core communication on device.

Key algebraic restructure vs the straightforward version: since the
per-pixel out einsum and Wo are both linear, fold them:
    y = sum_j Wo_j @ (ctx_norm_j^T @ q) = (sum_j Wo_j @ ctx_norm_j) @ q
      = WF @ q
where WF is a per-head-block [256, 32] matrix built once from the tiny
ctx blocks.  This turns the whole output phase into a single
[256x128] @ [128xN] matmul stream and removes the concat buffer and its
per-pixel zk normalization entirely (zk folds into WF).

zk (sum over pixels of exp(k)) is obtained for free by appending a
ones-column to the streamed v operand of the ctx matmul: ctx is computed
as ek^T-stationary x [v | 1], so column D of the ctx PSUM block is zk,
already transposed onto partitions.

Numerics: bf16 matmul inputs (host-cast), fp32 PSUM accumulation, fp32
scalar/vector math.  Softmaxes skip max-subtraction (logits ~N(0,1), exp
is safe in fp32).

SBUF layout: tensors with >128 channels are stored as [128, k*cols] with
128-channel k-tiles side by side in the free dim.  Key/value tensors are
kept in transposed [pixel, channel] layout (needed for the ctx einsum,
whose contraction runs over pixels); v tiles are 129 wide (128 channels
+ a ones column).
"""

import numpy as np
import ml_dtypes

import concourse.bass as bass
import concourse.bacc as bacc
import concourse.tile as tile
from concourse import mybir
from concourse.bass_utils import run_bass_kernel_spmd

BF16 = mybir.dt.bfloat16
F32 = mybir.dt.float32
AF = mybir.ActivationFunctionType

B, C, H, W = 4, 256, 128, 128
K, D = 8, 32
N = H * W
N_CORES = 8


def build_nc(n_loc=N):
    nc = bacc.Bacc("TRN2", target_bir_lowering=False, debug=False,
                   num_devices=N_CORES)

    nt = n_loc // 128        # 128-pixel tiles (128)
    nt512 = n_loc // 512     # 512-pixel tiles (32)

    # ---- I/O (weights pre-transposed, head-group-sliced, k-tiled on host) --
    # x/r/y are chunk-pair-major [npair*128, 2048]: pair pr rows hold pixels
    # 1024pr..1024pr+1024, col = 1024*(ch%2) + 512k + c.  Each paired DMA is
    # one fully contiguous [128, 2048] block -> 4 KiB packets per partition
    # row (the DMA engines' per-packet overhead dominates below ~4 KiB).
    npair = n_loc // 1024
    x = nc.declare_dram_parameter("x", [npair * 128, 2048], BF16,
                                  isOutput=False)
    r1 = nc.declare_dram_parameter("r1", [npair * 128, 2048], BF16,
                                   isOutput=False)
    r2 = nc.declare_dram_parameter("r2", [npair * 128, 2048], BF16,
                                   isOutput=False)
    # wq: [128, 2*128]  col chunk 128k = Wq.T[128k:128k+128, our 128 channels]
    wq = nc.declare_dram_parameter("wq", [128, 2 * 128], BF16, isOutput=False)
    # wkv_j: [128, 2*256] col chunk 256k = [WkT | WvT](our heads)[128k:, :]
    wkv1 = nc.declare_dram_parameter("wkv1", [128, 2 * 256], BF16, isOutput=False)
    wkv2 = nc.declare_dram_parameter("wkv2", [128, 2 * 256], BF16, isOutput=False)
    # wo: [128, 2*256]  col chunk 256j = Wo.T[(branch j, our heads) rows, :]
    wo = nc.declare_dram_parameter("wo", [128, 2 * 256], BF16, isOutput=False)
    ones4 = nc.declare_dram_parameter("ones4", [128, 32], BF16, isOutput=False)
    ones4T = nc.declare_dram_parameter("ones4T", [128, 128], BF16, isOutput=False)

    y = nc.declare_dram_parameter("y", [npair * 128, 2048], BF16,
                                  isOutput=True)

    refs = [r1, r2]

    with tile.TileContext(nc) as tc:
        with (
            tc.tile_pool(name="weights", bufs=1) as wpool,
            tc.tile_pool(name="persist", bufs=1) as ppool,
            tc.tile_pool(name="io", bufs=6) as iopool,
            tc.tile_pool(name="work", bufs=3) as wkpool,
        ):
            # ---- weights / constants.  Only wkv1 + ones4 are loaded up
            # front; the rest are issued between the first chunk DMAs so the
            # input stream starts as early as possible.
            wkv_t = [wpool.tile([128, 2 * 256], BF16, tag=f"wkv{j}",
                                name=f"wkv_t{j}") for j in range(2)]
            nc.gpsimd.dma_start(wkv_t[0][:], wkv1[:, :])
            ones4_t = wpool.tile([128, 32], BF16, tag="o4")
            nc.sync.dma_start(ones4_t[:], ones4[:, :])
            wq_t = wpool.tile([128, 2 * 128], BF16, tag="wq")
            ones4T_t = wpool.tile([128, 128], BF16, tag="o4T")
            wo_t = wpool.tile([128, 2 * 256], BF16, tag="wo")

            expq = ppool.tile([128, n_loc], BF16, tag="expq")
            nzc = (nt512 + 3) // 4
            zqr = ppool.tile([128, 512 * nzc], BF16, tag="zqr")
            recips = ppool.tile([128, 2], F32, tag="recips")
            compact = ppool.tile([128, 64], BF16, tag="compact")
            wft_sb = ppool.tile([128, 256], BF16, tag="wft_sb")

            CH = 4               # kv: 128-pixel tiles per chunk
            nch = nt // CH       # 32 chunks per branch

            with (
                tc.tile_pool(name="kvstage", bufs=1) as kvpool,
                tc.tile_pool(name="psA", bufs=2, space="PSUM") as psA,
                tc.tile_pool(name="psAcc", bufs=1, space="PSUM") as psAcc,
                tc.tile_pool(name="psQ", bufs=2, space="PSUM") as psQ,
            ):
                ekt_all = kvpool.tile([128, nt * 128], BF16, tag="ekt_all")
                vt_all = kvpool.tile([128, nt * 129], BF16, tag="vt_all")
                vt_v = vt_all.rearrange("p (t c) -> p t c", c=129)
                # ones column per v tile (survives both branches: the per-
                # chunk v copies only touch cols 0:128 of each 129-block)
                nc.vector.memset(vt_v[:, :, 128:129], 1.0)
                # ctx accumulators: one FULL bank per branch (cols 0:129
                # used; col 128 = zk).  Separate banks let branch 0's WF
                # chain read its ctx while branch 1 is still accumulating —
                # a DVE read racing PE writes in the same PSUM bank is fatal.
                ctx_ps_b = [psAcc.tile([128, 512], F32, tag=f"ctx{j}",
                                       name=f"ctx{j}") for j in range(2)]

                r_hold = [None]

                def pass1(j, ch):
                    if ch % 2 == 0:
                        pr = ch // 2
                        r_hold[0] = iopool.tile(
                            [128, 2048], BF16, tag="rchunk",
                            name=f"r_{j}_{pr}")
                        src = refs[j][128 * pr:128 * (pr + 1), :]
                        if j == 0 and ch == 0:
                            # first pair: 4 quarter DMAs on both queues so
                            # the very first kv matmuls start sooner
                            for qi in range(4):
                                eng = nc.sync if qi % 2 == 0 else nc.gpsimd
                                eng.dma_start(
                                    r_hold[0][:, 512 * qi:512 * (qi + 1)],
                                    src[:, 512 * qi:512 * (qi + 1)])
                        else:
                            dma_eng = nc.sync if pr % 2 == 0 else nc.gpsimd
                            dma_eng.dma_start(r_hold[0][:], src)
                    r_t = r_hold[0]
                    off = 1024 * (ch % 2)
                    kv_ps = psA.tile([128, CH * 256], F32, tag="kv",
                                     name=f"kv_{j}_{ch}")
                    for t in range(CH):
                        for k in range(2):
                            nc.tensor.matmul(
                                kv_ps[:, 256 * t:256 * (t + 1)],
                                r_t[:, off + 512 * k + 128 * t:
                                       off + 512 * k + 128 * (t + 1)],
                                wkv_t[j][:, 256 * k:256 * (k + 1)],
                                start=(k == 0), stop=(k == 1),
                            )
                    ek_sl = ekt_all[:, ch * CH * 128:(ch + 1) * CH * 128]
                    nc.scalar.activation(
                        ek_sl.rearrange("p (t c) -> p t c", t=CH),
                        kv_ps[:].rearrange("p (t c) -> p t c", t=CH)[:, :, 0:128],
                        AF.Exp,
                    )
                    nc.vector.tensor_copy(
                        vt_v[:, ch * CH:(ch + 1) * CH, 0:128],
                        kv_ps[:].rearrange("p (t c) -> p t c", t=CH)[:, :, 128:256],
                    )

                def pass2(j, ch):
                    # ctx accumulation: ek-tile stationary, [v | 1] streamed.
                    # out[c, d] = sum_pix ek[pix, c] v[pix, d]; col 128 = zk.
                    for t in range(ch * CH, (ch + 1) * CH):
                        nc.tensor.matmul(
                            ctx_ps_b[j][:, 0:129],
                            ekt_all[:, 128 * t:128 * (t + 1)],
                            vt_all[:, 129 * t:129 * (t + 1)],
                            start=(t == 0), stop=(t == nt - 1),
                        )

                x_hold = [None]

                def qchunk(i):
                    base = i * 512
                    if i % 2 == 0:
                        pr = i // 2
                        x_hold[0] = iopool.tile([128, 2048], BF16,
                                                tag="xchunk", name=f"x_{pr}")
                        dma_eng = nc.gpsimd if pr % 2 == 0 else nc.sync
                        dma_eng.dma_start(
                            x_hold[0][:], x[128 * pr:128 * (pr + 1), :])
                    x_t = x_hold[0]
                    off = 1024 * (i % 2)
                    q_ps = psQ.tile([128, 512], F32, tag="q", name=f"q_{i}")
                    for k in range(2):
                        nc.tensor.matmul(
                            q_ps[:], wq_t[:, 128 * k:128 * (k + 1)],
                            x_t[:, off + 512 * k:off + 512 * (k + 1)],
                            start=(k == 0), stop=(k == 1),
                        )
                    nc.scalar.activation(
                        expq[:, base:base + 512], q_ps[:], AF.Exp)

                def zqgroup(tc4):
                    # zq = per-head sums of expq (4 col-tiled concurrent MMs),
                    # reciprocal, then matmul-broadcast back over the 32
                    # partitions of each head and normalize expq in place.
                    zq_ps = psQ.tile([128, 512], F32, tag="q", name=f"zq_{tc4}")
                    for u in range(4):
                        t = 4 * tc4 + u
                        nc.tensor.matmul(
                            zq_ps[32 * u:32 * u + 32, :], ones4_t[:],
                            expq[:, 512 * t:512 * (t + 1)],
                            start=True, stop=True,
                            tile_position=(0, 32 * u),
                        )
                    zq_f = wkpool.tile([128, 512], F32, tag="zq_f",
                                       name=f"zqf_{tc4}")
                    nc.vector.reciprocal_approx_fast(zq_f[:], zq_ps[:])
                    nc.vector.tensor_copy(
                        zqr[:, 512 * tc4:512 * (tc4 + 1)], zq_f[:])
                    for t in range(4 * tc4, 4 * tc4 + 4):
                        u = t % 4
                        zqb_ps = psQ.tile([128, 512], F32, tag="q",
                                          name=f"zqb_{t}")
                        nc.tensor.matmul(
                            zqb_ps[:], ones4T_t[32 * u:32 * u + 4, :],
                            zqr[32 * u:32 * u + 4,
                                512 * tc4:512 * (tc4 + 1)],
                            start=True, stop=True,
                            tile_position=(32 * u, 0),
                        )
                        nc.vector.tensor_mul(
                            expq[:, 512 * t:512 * (t + 1)],
                            expq[:, 512 * t:512 * (t + 1)],
                            zqb_ps[:],
                        )

                # WF_j = WoT_j^T-blocks @ ctx_norm_j per head.  Branch 0's
                # half runs in branch 1's shadow — safe now that each
                # branch's ctx lives in its own PSUM bank.  The tiny WF
                # matmul scratch borrows a rotating psQ buffer (it is copied
                # to SBUF immediately, so the bank recycles right away).
                wf0_sb = ppool.tile([128, 256], F32, tag="wf0_sb")

                def wf_part(j):
                    nc.vector.reciprocal_approx_fast(
                        recips[:, j:j + 1], ctx_ps_b[j][:, 128:129])
                    for h in range(4):
                        nc.vector.tensor_scalar_mul(
                            compact[32 * h:32 * (h + 1), 32 * j:32 * j + 32],
                            ctx_ps_b[j][32 * h:32 * (h + 1),
                                        32 * h:32 * (h + 1)],
                            recips[32 * h:32 * (h + 1), j:j + 1],
                        )
                    wfq = psQ.tile([128, 512], F32, tag="q", name=f"wf_{j}")
                    for h in range(4):
                        nc.tensor.matmul(
                            wfq[32 * h:32 * (h + 1), 0:256],
                            compact[32 * h:32 * (h + 1), 32 * j:32 * j + 32],
                            wo_t[32 * h:32 * (h + 1), 256 * j:256 * (j + 1)],
                            start=True, stop=True,
                            tile_position=(32 * h, 32 * h),
                        )
                    if j == 0:
                        nc.vector.tensor_copy(wf0_sb[:], wfq[:, 0:256])
                    else:
                        nc.vector.tensor_add(
                            wft_sb[:], wf0_sb[:], wfq[:, 0:256])

                # ---- branches; q projection spread across both to even out
                # the DMA load (r + x/2 per branch).  Remaining weight DMAs
                # are dripped in between the chunk DMAs. ----
                # qchunks: 8 spread over ch 1..15, 8 packed into ch 16..23,
                # so the last zqgroup (and with it the whole q-normalize
                # tail) finishes ~5 chunks before the branch ends instead of
                # sitting serially on the critical path into phase C.
                ZQ_AT = {10: 0, 17: 1, 21: 2, 26: 3}
                for j in range(2):
                    for ch in range(nch):
                        pass1(j, ch)
                        if j == 0 and ch == 1:
                            nc.gpsimd.dma_start(wq_t[:], wq[:, :])
                        if j == 0 and ch == 5:
                            nc.sync.dma_start(ones4T_t[:], ones4T[:, :])
                        if j == 0 and ch == 8:
                            nc.sync.dma_start(wkv_t[1][:], wkv2[:, :])
                        if j == 0 and ch == 10:
                            nc.gpsimd.dma_start(wo_t[:], wo[:, :])
                        if ch > 0:
                            pass2(j, ch - 1)
                        if ch % 2 == 1 and ch <= 15:
                            qchunk(16 * j + (ch - 1) // 2)
                        elif 16 <= ch <= 23:
                            qchunk(16 * j + 8 + (ch - 16))
                        if ch in ZQ_AT:
                            zqgroup(4 * j + ZQ_AT[ch])
                    pass2(j, nch - 1)
                    wf_part(j)

            # ======= Phase C: y = WF @ expq, streamed over pixel tiles ======
            with (
                tc.tile_pool(name="psC", bufs=3, space="PSUM") as psC,
                tc.tile_pool(name="ysb", bufs=5) as ysbpool,
            ):
                y_hold = [None]
                for t in range(nt512):
                    y_ps = psC.tile([128, 1024], F32, tag="y", name=f"y_{t}")
                    for m in range(2):
                        nc.tensor.matmul(
                            y_ps[:, 512 * m:512 * (m + 1)],
                            wft_sb[:, 128 * m:128 * (m + 1)],
                            expq[:, 512 * t:512 * (t + 1)],
                            start=True, stop=True,
                        )
                    if t % 2 == 0:
                        y_hold[0] = ysbpool.tile([128, 2048], BF16, tag="ysb",
                                                 name=f"ysb_{t // 2}")
                    y_sb = y_hold[0]
                    sl = y_sb[:, 1024 * (t % 2):1024 * (t % 2) + 1024]
                    if t % 2 == 0:
                        nc.vector.tensor_copy(sl, y_ps[:])
                        if t == 0:
                            # kick the output stream one cast earlier: the
                            # out-DMA drain sets phase C's end time
                            nc.sync.dma_start(y[0:128, 0:1024],
                                              y_sb[:, 0:1024])
                    else:
                        nc.scalar.copy(sl, y_ps[:])
                        pr = t // 2
                        if t == 1:
                            nc.gpsimd.dma_start(y[0:128, 1024:2048],
                                                y_sb[:, 1024:2048])
                        else:
                            dma_eng = nc.sync if pr % 2 == 0 else nc.gpsimd
                            dma_eng.dma_start(
                                y[128 * pr:128 * (pr + 1), :], y_sb[:])

    nc.compile()
    return nc


def _consts():
    ones4 = np.zeros((128, 32), dtype=ml_dtypes.bfloat16)
    for col in range(32):
        a = col % 4
        ones4[32 * a:32 * (a + 1), col] = 1
    ones4T = np.zeros((128, 128), dtype=ml_dtypes.bfloat16)
    for u in range(4):
        for a in range(4):
            ones4T[32 * u + a, 32 * a:32 * (a + 1)] = 1
    return ones4, ones4T


def _ktile(wT):
    """[C_in, C_out] -> [128, (C_in//128)*C_out] k-tiles along the free dim."""
    kin = wT.shape[0] // 128
    return np.concatenate([wT[128 * k:128 * (k + 1), :] for k in range(kin)], axis=1)


def _chunkmajor(arr, n_loc=N):
    """[256, n] -> [npair*128, 2048]: pair pr holds pixels 1024pr..1024pr+1024
    at col = 1024*(ch%2) + 512k + c (k = channel k-tile)."""
    npair = n_loc // 1024
    # [k, p, pr, e, c] -> [pr, p, e, k, c]
    a = arr.reshape(2, 128, npair, 2, 512).transpose(2, 1, 3, 0, 4)
    return a.reshape(npair * 128, 2048)


def _unchunk_y(yarr, n_loc=N):
    """[npair*128, 2048] -> [256, n]: per pair pr, col 1024e + 512m + c of
    partition p is y[128m + p, 512*(2pr + e) + c]."""
    npair = n_loc // 1024
    # [pr, p, e, m, c] -> [m, p, pr, e, c]
    a = yarr.reshape(npair, 128, 2, 2, 512).transpose(3, 1, 0, 2, 4)
    return a.reshape(C, n_loc)


def make_in_maps(x, ref_1, ref_2, Wq, Wk1, Wk2, Wv1, Wv2, Wo, n_loc=N):
    bf = ml_dtypes.bfloat16
    ones4, ones4T = _consts()
    xf = np.asarray(x).reshape(B, C, -1)
    r1f = np.asarray(ref_1).reshape(B, C, -1)
    r2f = np.asarray(ref_2).reshape(B, C, -1)
    WqT, WoT = np.asarray(Wq).T, np.asarray(Wo).T
    WkT = [np.asarray(Wk1).T, np.asarray(Wk2).T]
    WvT = [np.asarray(Wv1).T, np.asarray(Wv2).T]
    gw = {}
    for g in range(2):
        sl = slice(128 * g, 128 * (g + 1))
        wq_g = np.ascontiguousarray(_ktile(WqT[:, sl])).astype(bf)
        wkv_g = [np.ascontiguousarray(
            _ktile(np.concatenate([WkT[j][:, sl], WvT[j][:, sl]], axis=1))
        ).astype(bf) for j in range(2)]
        # Wo rows for our concat channels: branch j block at cols 256j
        wo_g = np.ascontiguousarray(np.concatenate(
            [WoT[128 * g:128 * (g + 1), :],
             WoT[256 + 128 * g:256 + 128 * (g + 1), :]],
            axis=1)).astype(bf)
        gw[g] = (wq_g, wkv_g[0], wkv_g[1], wo_g)
    in_maps = []
    for core in range(N_CORES):
        b, g = core // 2, core % 2
        wq_g, wkv1_g, wkv2_g, wo_g = gw[g]
        in_maps.append({
            "x": np.ascontiguousarray(_chunkmajor(xf[b, :, :n_loc], n_loc)).astype(bf),
            "r1": np.ascontiguousarray(_chunkmajor(r1f[b, :, :n_loc], n_loc)).astype(bf),
            "r2": np.ascontiguousarray(_chunkmajor(r2f[b, :, :n_loc], n_loc)).astype(bf),
            "wq": wq_g, "wkv1": wkv1_g, "wkv2": wkv2_g, "wo": wo_g,
            "ones4": ones4, "ones4T": ones4T,
        })
    return in_maps


_NC_CACHE = {}


def kernel(x, ref_1, ref_2, Wq, Wk1, Wk2, Wv1, Wv2, Wo, _trace=False):
    n_loc = N
    if n_loc not in _NC_CACHE:
        _NC_CACHE[n_loc] = build_nc(n_loc)
    nc = _NC_CACHE[n_loc]
    in_maps = make_in_maps(x, ref_1, ref_2, Wq, Wk1, Wk2, Wv1, Wv2, Wo, n_loc)
    res = run_bass_kernel_spmd(nc, in_maps, core_ids=list(range(N_CORES)),
                               trace=_trace)
    out = np.empty((B, C, n_loc), dtype=np.float32)
    for b in range(B):
        out[b] = (_unchunk_y(res.results[2 * b]["y"].astype(np.float32), n_loc)
                  + _unchunk_y(res.results[2 * b + 1]["y"].astype(np.float32), n_loc))
    if _trace:
        kernel.last_results = res
    return out.reshape(B, C, H, W)


# revision 51
# speedup vs baseline: 1.1662x; 1.0120x over previous
"""Bidirectional cross-attention kernel for Trainium2, SPMD over 8 NeuronCores.

Reference (per batch b, heads K=8, head dim D=32, N=128*128 pixels):
    q   = softmax_d(Wq @ x)
    for branch j in {1,2}:
        key   = softmax_n(Wk_j @ ref_j)          # softmax over the pixel dim
        v     = Wv_j @ ref_j
        ctx_j = key @ v^T                        # [K,D,D]
        out_j = per-pixel  q @ ctx_j^T
    y = Wo @ concat(out_1, out_2)

Sharding: 8 cores = batch(4) x head-group(2).  Each core owns 4 of the 8
heads for its batch: projections, softmaxes, ctx and the out einsum are
fully head-local; the final Wo projection is computed as a partial sum
over the core's 256 (of 512) concat channels, and the host adds the two
partial outputs per batch.  No cross-core communication on device.

Key algebraic restructure vs the straightforward version: since the
per-pixel out einsum and Wo are both linear, fold them:
    y = sum_j Wo_j @ (ctx_norm_j^T @ q) = (sum_j Wo_j @ ctx_norm_j) @ q
      = WF @ q
where WF is a per-head-block [256, 32] matrix built once from the tiny
ctx blocks.  This turns the whole output phase into a single
[256x128] @ [128xN] matmul stream and removes the concat buffer and its
per-pixel zk normalization entirely (zk folds into WF).

zk (sum over pixels of exp(k)) is obtained for free by appending a
ones-column to the streamed v operand of the ctx matmul: ctx is computed
as ek^T-stationary x [v | 1], so column D of the ctx PSUM block is zk,
already transposed onto partitions.

Numerics: bf16 matmul inputs (host-cast), fp32 PSUM accumulation, fp32
scalar/vector math.  Softmaxes skip max-subtraction (logits ~N(0,1), exp
is safe in fp32).

SBUF layout: tensors with >128 channels are stored as [128, k*cols] with
128-channel k-tiles side by side in the free dim.  Key/value tensors are
kept in transposed [pixel, channel] layout (needed for the ctx einsum,
whose contraction runs over pixels); v tiles are 129 wide (128 channels
+ a ones column).
"""

import numpy as np
import ml_dtypes

import concourse.bass as bass
import concourse.bacc as bacc
import concourse.tile as tile
from concourse import mybir
from concourse.bass_utils import run_bass_kernel_spmd

BF16 = mybir.dt.bfloat16
F32 = mybir.dt.float32
AF = mybir.ActivationFunctionType

B, C, H, W = 4, 256, 128, 128
K, D = 8, 32
N = H * W
N_CORES = 8


def build_nc(n_loc=N):
    nc = bacc.Bacc("TRN2", target_bir_lowering=False, debug=False,
                   num_devices=N_CORES)

    nt = n_loc // 128        # 128-pixel tiles (128)
    nt512 = n_loc // 512     # 512-pixel tiles (32)

    # ---- I/O (weights pre-transposed, head-group-sliced, k-tiled on host) --
    # x/r/y are chunk-pair-major [npair*128, 2048]: pair pr rows hold pixels
    # 1024pr..1024pr+1024, col = 1024*(ch%2) + 512k + c.  Each paired DMA is
    # one fully contiguous [128, 2048] block -> 4 KiB packets per partition
    # row (the DMA engines' per-packet overhead dominates below ~4 KiB).
    npair = n_loc // 1024
    x = nc.declare_dram_parameter("x", [npair * 128, 2048], BF16,
                                  isOutput=False)
    r1 = nc.declare_dram_parameter("r1", [npair * 128, 2048], BF16,
                                   isOutput=False)
    r2 = nc.declare_dram_parameter("r2", [npair * 128, 2048], BF16,
                                   isOutput=False)
    # wq: [128, 2*128]  col chunk 128k = Wq.T[128k:128k+128, our 128 channels]
    wq = nc.declare_dram_parameter("wq", [128, 2 * 128], BF16, isOutput=False)
    # wkv_j: [128, 2*256] col chunk 256k = [WkT | WvT](our heads)[128k:, :]
    wkv1 = nc.declare_dram_parameter("wkv1", [128, 2 * 256], BF16, isOutput=False)
    wkv2 = nc.declare_dram_parameter("wkv2", [128, 2 * 256], BF16, isOutput=False)
    # wo: [128, 2*256]  col chunk 256j = Wo.T[(branch j, our heads) rows, :]
    wo = nc.declare_dram_parameter("wo", [128, 2 * 256], BF16, isOutput=False)
    ones4 = nc.declare_dram_parameter("ones4", [128, 32], BF16, isOutput=False)
    ones4T = nc.declare_dram_parameter("ones4T", [128, 128], BF16, isOutput=False)

    y = nc.declare_dram_parameter("y", [npair * 128, 2048], BF16,
                                  isOutput=True)

    refs = [r1, r2]

    with tile.TileContext(nc) as tc:
        with (
            tc.tile_pool(name="weights", bufs=1) as wpool,
            tc.tile_pool(name="persist", bufs=1) as ppool,
            tc.tile_pool(name="io", bufs=6) as iopool,
            tc.tile_pool(name="work", bufs=3) as wkpool,
        ):
            # ---- weights / constants.  Only wkv1 + ones4 are loaded up
            # front; the rest are issued between the first chunk DMAs so the
            # input stream starts as early as possible.
            wkv_t = [wpool.tile([128, 2 * 256], BF16, tag=f"wkv{j}",
                                name=f"wkv_t{j}") for j in range(2)]
            nc.gpsimd.dma_start(wkv_t[0][:], wkv1[:, :])
            ones4_t = wpool.tile([128, 32], BF16, tag="o4")
            nc.sync.dma_start(ones4_t[:], ones4[:, :])
            wq_t = wpool.tile([128, 2 * 128], BF16, tag="wq")
            ones4T_t = wpool.tile([128, 128], BF16, tag="o4T")
            wo_t = wpool.tile([128, 2 * 256], BF16, tag="wo")

            expq = ppool.tile([128, n_loc], BF16, tag="expq")
            nzc = (nt512 + 3) // 4
            zqr = ppool.tile([128, 512 * nzc], BF16, tag="zqr")
            recips = ppool.tile([128, 2], F32, tag="recips")
            compact = ppool.tile([128, 64], BF16, tag="compact")
            wft_sb = ppool.tile([128, 256], BF16, tag="wft_sb")

            CH = 4               # kv: 128-pixel tiles per chunk
            nch = nt // CH       # 32 chunks per branch

            with (
                tc.tile_pool(name="kvstage", bufs=1) as kvpool,
                tc.tile_pool(name="psA", bufs=2, space="PSUM") as psA,
                tc.tile_pool(name="psAcc", bufs=1, space="PSUM") as psAcc,
                tc.tile_pool(name="psQ", bufs=2, space="PSUM") as psQ,
            ):
                ekt_all = kvpool.tile([128, nt * 128], BF16, tag="ekt_all")
                vt_all = kvpool.tile([128, nt * 129], BF16, tag="vt_all")
                vt_v = vt_all.rearrange("p (t c) -> p t c", c=129)
                # ones column per v tile (survives both branches: the per-
                # chunk v copies only touch cols 0:128 of each 129-block)
                nc.vector.memset(vt_v[:, :, 128:129], 1.0)
                # ctx accumulators: one FULL bank per branch (cols 0:129
                # used; col 128 = zk).  Separate banks let branch 0's WF
                # chain read its ctx while branch 1 is still accumulating —
                # a DVE read racing PE writes in the same PSUM bank is fatal.
                ctx_ps_b = [psAcc.tile([128, 512], F32, tag=f"ctx{j}",
                                       name=f"ctx{j}") for j in range(2)]

                r_hold = [None]

                def pass1(j, ch):
                    if ch % 2 == 0:
                        pr = ch // 2
                        r_hold[0] = iopool.tile(
                            [128, 2048], BF16, tag="rchunk",
                            name=f"r_{j}_{pr}")
                        src = refs[j][128 * pr:128 * (pr + 1), :]
                        if j == 0 and ch == 0:
                            # first pair: 4 quarter DMAs on both queues so
                            # the very first kv matmuls start sooner
                            for qi in range(4):
                                eng = nc.sync if qi % 2 == 0 else nc.gpsimd
                                eng.dma_start(
                                    r_hold[0][:, 512 * qi:512 * (qi + 1)],
                                    src[:, 512 * qi:512 * (qi + 1)])
                        else:
                            dma_eng = nc.sync if pr % 2 == 0 else nc.gpsimd
                            dma_eng.dma_start(r_hold[0][:], src)
                    r_t = r_hold[0]
                    off = 1024 * (ch % 2)
                    kv_ps = psA.tile([128, CH * 256], F32, tag="kv",
                                     name=f"kv_{j}_{ch}")
                    for t in range(CH):
                        for k in range(2):
                            nc.tensor.matmul(
                                kv_ps[:, 256 * t:256 * (t + 1)],
                                r_t[:, off + 512 * k + 128 * t:
                                       off + 512 * k + 128 * (t + 1)],
                                wkv_t[j][:, 256 * k:256 * (k + 1)],
                                start=(k == 0), stop=(k == 1),
                            )
                    ek_sl = ekt_all[:, ch * CH * 128:(ch + 1) * CH * 128]
                    nc.scalar.activation(
                        ek_sl.rearrange("p (t c) -> p t c", t=CH),
                        kv_ps[:].rearrange("p (t c) -> p t c", t=CH)[:, :, 0:128],
                        AF.Exp,
                    )
                    nc.vector.tensor_copy(
                        vt_v[:, ch * CH:(ch + 1) * CH, 0:128],
                        kv_ps[:].rearrange("p (t c) -> p t c", t=CH)[:, :, 128:256],
                    )

                def pass2(j, ch):
                    # ctx accumulation: ek-tile stationary, [v | 1] streamed.
                    # out[c, d] = sum_pix ek[pix, c] v[pix, d]; col 128 = zk.
                    for t in range(ch * CH, (ch + 1) * CH):
                        nc.tensor.matmul(
                            ctx_ps_b[j][:, 0:129],
                            ekt_all[:, 128 * t:128 * (t + 1)],
                            vt_all[:, 129 * t:129 * (t + 1)],
                            start=(t == 0), stop=(t == nt - 1),
                        )

                x_hold = [None]

                def qchunk(i):
                    base = i * 512
                    if i % 2 == 0:
                        pr = i // 2
                        x_hold[0] = iopool.tile([128, 2048], BF16,
                                                tag="xchunk", name=f"x_{pr}")
                        dma_eng = nc.gpsimd if pr % 2 == 0 else nc.sync
                        dma_eng.dma_start(
                            x_hold[0][:], x[128 * pr:128 * (pr + 1), :])
                    x_t = x_hold[0]
                    off = 1024 * (i % 2)
                    q_ps = psQ.tile([128, 512], F32, tag="q", name=f"q_{i}")
                    for k in range(2):
                        nc.tensor.matmul(
                            q_ps[:], wq_t[:, 128 * k:128 * (k + 1)],
                            x_t[:, off + 512 * k:off + 512 * (k + 1)],
                            start=(k == 0), stop=(k == 1),
                        )
                    nc.scalar.activation(
                        expq[:, base:base + 512], q_ps[:], AF.Exp)

                def zqgroup(tc4):
                    # zq = per-head sums of expq (4 col-tiled concurrent MMs),
                    # reciprocal, then matmul-broadcast back over the 32
                    # partitions of each head and normalize expq in place.
                    zq_ps = psQ.tile([128, 512], F32, tag="q", name=f"zq_{tc4}")
                    for u in range(4):
                        t = 4 * tc4 + u
                        nc.tensor.matmul(
                            zq_ps[32 * u:32 * u + 32, :], ones4_t[:],
                            expq[:, 512 * t:512 * (t + 1)],
                            start=True, stop=True,
                            tile_position=(0, 32 * u),
                        )
                    zq_f = wkpool.tile([128, 512], F32, tag="zq_f",
                                       name=f"zqf_{tc4}")
                    nc.vector.reciprocal_approx_fast(zq_f[:], zq_ps[:])
                    nc.vector.tensor_copy(
                        zqr[:, 512 * tc4:512 * (tc4 + 1)], zq_f[:])
                    for t in range(4 * tc4, 4 * tc4 + 4):
                        u = t % 4
                        zqb_ps = psQ.tile([128, 512], F32, tag="q",
                                          name=f"zqb_{t}")
                        nc.tensor.matmul(
                            zqb_ps[:], ones4T_t[32 * u:32 * u + 4, :],
                            zqr[32 * u:32 * u + 4,
                                512 * tc4:512 * (tc4 + 1)],
                            start=True, stop=True,
                            tile_position=(32 * u, 0),
                        )
                        nc.vector.tensor_mul(
                            expq[:, 512 * t:512 * (t + 1)],
                            expq[:, 512 * t:512 * (t + 1)],
                            zqb_ps[:],
                        )

                # WF_j = WoT_j^T-blocks @ ctx_norm_j per head.  Branch 0's
                # half runs in branch 1's shadow — safe now that each
                # branch's ctx lives in its own PSUM bank.  The tiny WF
                # matmul scratch borrows a rotating psQ buffer (it is copied
                # to SBUF immediately, so the bank recycles right away).
                wf0_sb = ppool.tile([128, 256], F32, tag="wf0_sb")

                def wf_part(j):
                    nc.vector.reciprocal_approx_fast(
                        recips[:, j:j + 1], ctx_ps_b[j][:, 128:129])
                    for h in range(4):
                        nc.vector.tensor_scalar_mul(
                            compact[32 * h:32 * (h + 1), 32 * j:32 * j + 32],
                            ctx_ps_b[j][32 * h:32 * (h + 1),
                                        32 * h:32 * (h + 1)],
                            recips[32 * h:32 * (h + 1), j:j + 1],
                        )
                    wfq = psQ.tile([128, 512], F32, tag="q", name=f"wf_{j}")
                    for h in range(4):
                        nc.tensor.matmul(
                            wfq[32 * h:32 * (h + 1), 0:256],
                            compact[32 * h:32 * (h + 1), 32 * j:32 * j + 32],
                            wo_t[32 * h:32 * (h + 1), 256 * j:256 * (j + 1)],
                            start=True, stop=True,
                            tile_position=(32 * h, 32 * h),
                        )
                    if j == 0:
                        nc.vector.tensor_copy(wf0_sb[:], wfq[:, 0:256])
                    else:
                        nc.vector.tensor_add(
                            wft_sb[:], wf0_sb[:], wfq[:, 0:256])

                # ---- branches; q projection spread across both to even out
                # the DMA load (r + x/2 per branch).  Remaining weight DMAs
                # are dripped in between the chunk DMAs. ----
                # qchunks: 8 spread over ch 3..17, 8 packed into ch 18..25,
                # so (i) the first two chunk-pairs of r get the whole DMA
                # ramp to themselves (the early kv stalls were r-pairs
                # arriving late while x pair 0 competed), and (ii) the last
                # zqgroup still finishes ~5 chunks before the branch ends.
                ZQ_AT = {10: 0, 18: 1, 22: 2, 26: 3}
                for j in range(2):
                    for ch in range(nch):
                        pass1(j, ch)
                        if j == 0 and ch == 1:
                            nc.gpsimd.dma_start(wq_t[:], wq[:, :])
                        if j == 0 and ch == 5:
                            nc.sync.dma_start(ones4T_t[:], ones4T[:, :])
                        if j == 0 and ch == 8:
                            nc.sync.dma_start(wkv_t[1][:], wkv2[:, :])
                        if j == 0 and ch == 10:
                            nc.gpsimd.dma_start(wo_t[:], wo[:, :])
                        if ch > 0:
                            pass2(j, ch - 1)
                        if ch % 2 == 1 and 3 <= ch <= 17:
                            qchunk(16 * j + (ch - 3) // 2)
                        elif 18 <= ch <= 25:
                            qchunk(16 * j + 8 + (ch - 18))
                        if ch in ZQ_AT:
                            zqgroup(4 * j + ZQ_AT[ch])
                    pass2(j, nch - 1)
                    wf_part(j)

            # ======= Phase C: y = WF @ expq, streamed over pixel tiles ======
            with (
                tc.tile_pool(name="psC", bufs=4, space="PSUM") as psC,
                tc.tile_pool(name="ysb", bufs=6) as ysbpool,
            ):
                y_hold = [None]
                for t in range(nt512):
                    y_ps = psC.tile([128, 1024], F32, tag="y", name=f"y_{t}")
                    for m in range(2):
                        nc.tensor.matmul(
                            y_ps[:, 512 * m:512 * (m + 1)],
                            wft_sb[:, 128 * m:128 * (m + 1)],
                            expq[:, 512 * t:512 * (t + 1)],
                            start=True, stop=True,
                        )
                    if t % 2 == 0:
                        y_hold[0] = ysbpool.tile([128, 2048], BF16, tag="ysb",
                                                 name=f"ysb_{t // 2}")
                    y_sb = y_hold[0]
                    sl = y_sb[:, 1024 * (t % 2):1024 * (t % 2) + 1024]
                    if t % 2 == 0:
                        nc.vector.tensor_copy(sl, y_ps[:])
                        if t == 0:
                            # kick the output stream one cast earlier: the
                            # out-DMA drain sets phase C's end time
                            nc.sync.dma_start(y[0:128, 0:1024],
                                              y_sb[:, 0:1024])
                    else:
                        nc.scalar.copy(sl, y_ps[:])
                        pr = t // 2
                        if t == 1:
                            nc.gpsimd.dma_start(y[0:128, 1024:2048],
                                                y_sb[:, 1024:2048])
                        else:
                            dma_eng = nc.sync if pr % 2 == 0 else nc.gpsimd
                            dma_eng.dma_start(
                                y[128 * pr:128 * (pr + 1), :], y_sb[:])

    nc.compile()
    return nc


def _consts():
    ones4 = np.zeros((128, 32), dtype=ml_dtypes.bfloat16)
    for col in range(32):
        a = col % 4
        ones4[32 * a:32 * (a + 1), col] = 1
    ones4T = np.zeros((128, 128), dtype=ml_dtypes.bfloat16)
    for u in range(4):
        for a in range(4):
            ones4T[32 * u + a, 32 * a:32 * (a + 1)] = 1
    return ones4, ones4T


def _ktile(wT):
    """[C_in, C_out] -> [128, (C_in//128)*C_out] k-tiles along the free dim."""
    kin = wT.shape[0] // 128
    return np.concatenate([wT[128 * k:128 * (k + 1), :] for k in range(kin)], axis=1)


def _chunkmajor(arr, n_loc=N):
    """[256, n] -> [npair*128, 2048]: pair pr holds pixels 1024pr..1024pr+1024
    at col = 1024*(ch%2) + 512k + c (k = channel k-tile)."""
    npair = n_loc // 1024
    # [k, p, pr, e, c] -> [pr, p, e, k, c]
    a = arr.reshape(2, 128, npair, 2, 512).transpose(2, 1, 3, 0, 4)
    return a.reshape(npair * 128, 2048)


def _unchunk_y(yarr, n_loc=N):
    """[npair*128, 2048] -> [256, n]: per pair pr, col 1024e + 512m + c of
    partition p is y[128m + p, 512*(2pr + e) + c]."""
    npair = n_loc // 1024
    # [pr, p, e, m, c] -> [m, p, pr, e, c]
    a = yarr.reshape(npair, 128, 2, 2, 512).transpose(3, 1, 0, 2, 4)
    return a.reshape(C, n_loc)


def make_in_maps(x, ref_1, ref_2, Wq, Wk1, Wk2, Wv1, Wv2, Wo, n_loc=N):
    bf = ml_dtypes.bfloat16
    ones4, ones4T = _consts()
    xf = np.asarray(x).reshape(B, C, -1)
    r1f = np.asarray(ref_1).reshape(B, C, -1)
    r2f = np.asarray(ref_2).reshape(B, C, -1)
    WqT, WoT = np.asarray(Wq).T, np.asarray(Wo).T
    WkT = [np.asarray(Wk1).T, np.asarray(Wk2).T]
    WvT = [np.asarray(Wv1).T, np.asarray(Wv2).T]
    gw = {}
    for g in range(2):
        sl = slice(128 * g, 128 * (g + 1))
        wq_g = np.ascontiguousarray(_ktile(WqT[:, sl])).astype(bf)
        wkv_g = [np.ascontiguousarray(
            _ktile(np.concatenate([WkT[j][:, sl], WvT[j][:, sl]], axis=1))
        ).astype(bf) for j in range(2)]
        # Wo rows for our concat channels: branch j block at cols 256j
        wo_g = np.ascontiguousarray(np.concatenate(
            [WoT[128 * g:128 * (g + 1), :],
             WoT[256 + 128 * g:256 + 128 * (g + 1), :]],
            axis=1)).astype(bf)
        gw[g] = (wq_g, wkv_g[0], wkv_g[1], wo_g)
    in_maps = []
    for core in range(N_CORES):
        b, g = core // 2, core % 2
        wq_g, wkv1_g, wkv2_g, wo_g = gw[g]
        in_maps.append({
            "x": np.ascontiguousarray(_chunkmajor(xf[b, :, :n_loc], n_loc)).astype(bf),
            "r1": np.ascontiguousarray(_chunkmajor(r1f[b, :, :n_loc], n_loc)).astype(bf),
            "r2": np.ascontiguousarray(_chunkmajor(r2f[b, :, :n_loc], n_loc)).astype(bf),
            "wq": wq_g, "wkv1": wkv1_g, "wkv2": wkv2_g, "wo": wo_g,
            "ones4": ones4, "ones4T": ones4T,
        })
    return in_maps


_NC_CACHE = {}


def kernel(x, ref_1, ref_2, Wq, Wk1, Wk2, Wv1, Wv2, Wo, _trace=False):
    n_loc = N
    if n_loc not in _NC_CACHE:
        _NC_CACHE[n_loc] = build_nc(n_loc)
    nc = _NC_CACHE[n_loc]
    in_maps = make_in_maps(x, ref_1, ref_2, Wq, Wk1, Wk2, Wv1, Wv2, Wo, n_loc)
    res = run_bass_kernel_spmd(nc, in_maps, core_ids=list(range(N_CORES)),
                               trace=_trace)
    out = np.empty((B, C, n_loc), dtype=np.float32)
    for b in range(B):
        out[b] = (_unchunk_y(res.results[2 * b]["y"].astype(np.float32), n_loc)
                  + _unchunk_y(res.results[2 * b + 1]["y"].astype(np.float32), n_loc))
    if _trace:
        kernel.last_results = res
    return out.reshape(B, C, H, W)


# revision 52
# speedup vs baseline: 1.2199x; 1.0460x over previous
"""Bidirectional cross-attention kernel for Trainium2, SPMD over 8 NeuronCores.

Reference (per batch b, heads K=8, head dim D=32, N=128*128 pixels):
    q   = softmax_d(Wq @ x)
    for branch j in {1,2}:
        key   = softmax_n(Wk_j @ ref_j)          # softmax over the pixel dim
        v     = Wv_j @ ref_j
        ctx_j = key @ v^T                        # [K,D,D]
        out_j = per-pixel  q @ ctx_j^T
    y = Wo @ concat(out_1, out_2)

Sharding: 8 cores = batch(4) x head-group(2).  Each core owns 4 of the 8
heads for its batch: projections, softmaxes, ctx and the out einsum are
fully head-local; the final Wo projection is computed as a partial sum
over the core's 256 (of 512) concat channels, and the host adds the two
partial outputs per batch.  No cross-core communication on device.

Key algebraic restructure vs the straightforward version: since the
per-pixel out einsum and Wo are both linear, fold them:
    y = sum_j Wo_j @ (ctx_norm_j^T @ q) = (sum_j Wo_j @ ctx_norm_j) @ q
      = WF @ q
where WF is a per-head-block [256, 32] matrix built once from the tiny
ctx blocks.  This turns the whole output phase into a single
[256x128] @ [128xN] matmul stream and removes the concat buffer and its
per-pixel zk normalization entirely (zk folds into WF).

zk (sum over pixels of exp(k)) is obtained for free by appending a
ones-column to the streamed v operand of the ctx matmul: ctx is computed
as ek^T-stationary x [v | 1], so column D of the ctx PSUM block is zk,
already transposed onto partitions.

Numerics: bf16 matmul inputs (host-cast), fp32 PSUM accumulation, fp32
scalar/vector math.  Softmaxes skip max-subtraction (logits ~N(0,1), exp
is safe in fp32).

SBUF layout: tensors with >128 channels are stored as [128, k*cols] with
128-channel k-tiles side by side in the free dim.  Key/value tensors are
kept in transposed [pixel, channel] layout (needed for the ctx einsum,
whose contraction runs over pixels); v tiles are 129 wide (128 channels
+ a ones column).
"""

import numpy as np
import ml_dtypes

import concourse.bass as bass
import concourse.bacc as bacc
import concourse.tile as tile
from concourse import mybir
from concourse.bass_utils import run_bass_kernel_spmd

BF16 = mybir.dt.bfloat16
F32 = mybir.dt.float32
AF = mybir.ActivationFunctionType

B, C, H, W = 4, 256, 128, 128
K, D = 8, 32
N = H * W
N_CORES = 8


def build_nc(n_loc=N):
    nc = bacc.Bacc("TRN2", target_bir_lowering=False, debug=False,
                   num_devices=N_CORES)

    nt = n_loc // 128        # 128-pixel tiles (128)
    nt512 = n_loc // 512     # 512-pixel tiles (32)

    # ---- I/O (weights pre-transposed, head-group-sliced, k-tiled on host) --
    # x/r/y are chunk-pair-major [npair*128, 2048]: pair pr rows hold pixels
    # 1024pr..1024pr+1024, col = 1024*(ch%2) + 512k + c.  Each paired DMA is
    # one fully contiguous [128, 2048] block -> 4 KiB packets per partition
    # row (the DMA engines' per-packet overhead dominates below ~4 KiB).
    npair = n_loc // 1024
    x = nc.declare_dram_parameter("x", [npair * 128, 2048], BF16,
                                  isOutput=False)
    r1 = nc.declare_dram_parameter("r1", [npair * 128, 2048], BF16,
                                   isOutput=False)
    r2 = nc.declare_dram_parameter("r2", [npair * 128, 2048], BF16,
                                   isOutput=False)
    # wq: [128, 2*128]  col chunk 128k = Wq.T[128k:128k+128, our 128 channels]
    wq = nc.declare_dram_parameter("wq", [128, 2 * 128], BF16, isOutput=False)
    # wkv_j: [128, 2*256] col chunk 256k = [WkT | WvT](our heads)[128k:, :]
    wkv1 = nc.declare_dram_parameter("wkv1", [128, 2 * 256], BF16, isOutput=False)
    wkv2 = nc.declare_dram_parameter("wkv2", [128, 2 * 256], BF16, isOutput=False)
    # wo: [128, 2*256]  col chunk 256j = Wo.T[(branch j, our heads) rows, :]
    wo = nc.declare_dram_parameter("wo", [128, 2 * 256], BF16, isOutput=False)
    ones4 = nc.declare_dram_parameter("ones4", [128, 32], BF16, isOutput=False)
    ones4T = nc.declare_dram_parameter("ones4T", [128, 128], BF16, isOutput=False)

    y = nc.declare_dram_parameter("y", [npair * 128, 2048], BF16,
                                  isOutput=True)

    refs = [r1, r2]

    with tile.TileContext(nc) as tc:
        with (
            tc.tile_pool(name="weights", bufs=1) as wpool,
            tc.tile_pool(name="persist", bufs=1) as ppool,
            tc.tile_pool(name="io", bufs=6) as iopool,
            tc.tile_pool(name="work", bufs=3) as wkpool,
        ):
            # ---- weights / constants.  Only wkv1 + ones4 are loaded up
            # front; the rest are issued between the first chunk DMAs so the
            # input stream starts as early as possible.
            wkv_t = [wpool.tile([128, 2 * 256], BF16, tag=f"wkv{j}",
                                name=f"wkv_t{j}") for j in range(2)]
            nc.gpsimd.dma_start(wkv_t[0][:], wkv1[:, :])
            ones4_t = wpool.tile([128, 32], BF16, tag="o4")
            nc.sync.dma_start(ones4_t[:], ones4[:, :])
            wq_t = wpool.tile([128, 2 * 128], BF16, tag="wq")
            ones4T_t = wpool.tile([128, 128], BF16, tag="o4T")
            wo_t = wpool.tile([128, 2 * 256], BF16, tag="wo")

            expq = ppool.tile([128, n_loc], BF16, tag="expq")
            nzc = (nt512 + 3) // 4
            zqr = ppool.tile([128, 512 * nzc], BF16, tag="zqr")
            recips = ppool.tile([128, 2], F32, tag="recips")
            compact = ppool.tile([128, 64], BF16, tag="compact")
            wft_sb = ppool.tile([128, 256], BF16, tag="wft_sb")

            CH = 4               # kv: 128-pixel tiles per chunk
            nch = nt // CH       # 32 chunks per branch

            with (
                tc.tile_pool(name="kvstage", bufs=1) as kvpool,
                tc.tile_pool(name="psA", bufs=2, space="PSUM") as psA,
                tc.tile_pool(name="psAcc", bufs=1, space="PSUM") as psAcc,
                tc.tile_pool(name="psQ", bufs=2, space="PSUM") as psQ,
            ):
                ekt_all = kvpool.tile([128, nt * 128], BF16, tag="ekt_all")
                vt_all = kvpool.tile([128, nt * 129], BF16, tag="vt_all")
                vt_v = vt_all.rearrange("p (t c) -> p t c", c=129)
                # ones column per v tile (survives both branches: the per-
                # chunk v copies only touch cols 0:128 of each 129-block)
                nc.vector.memset(vt_v[:, :, 128:129], 1.0)
                # ctx accumulators: one FULL bank per branch (cols 0:129
                # used; col 128 = zk).  Separate banks let branch 0's WF
                # chain read its ctx while branch 1 is still accumulating —
                # a DVE read racing PE writes in the same PSUM bank is fatal.
                ctx_ps_b = [psAcc.tile([128, 512], F32, tag=f"ctx{j}",
                                       name=f"ctx{j}") for j in range(2)]

                r_hold = [None]

                def pass1(j, ch):
                    if ch % 2 == 0:
                        pr = ch // 2
                        r_hold[0] = iopool.tile(
                            [128, 2048], BF16, tag="rchunk",
                            name=f"r_{j}_{pr}")
                        src = refs[j][128 * pr:128 * (pr + 1), :]
                        if j == 0 and ch == 0:
                            # first pair: 4 quarter DMAs on both queues so
                            # the very first kv matmuls start sooner
                            for qi in range(4):
                                eng = nc.sync if qi % 2 == 0 else nc.gpsimd
                                eng.dma_start(
                                    r_hold[0][:, 512 * qi:512 * (qi + 1)],
                                    src[:, 512 * qi:512 * (qi + 1)])
                        else:
                            dma_eng = nc.sync if pr % 2 == 0 else nc.gpsimd
                            dma_eng.dma_start(r_hold[0][:], src)
                    r_t = r_hold[0]
                    off = 1024 * (ch % 2)
                    # k-out and v-out in SEPARATE PSUM banks: the ek exp
                    # (Scalar) and v copy (Vector) then read different banks
                    # and run in parallel — same-bank PSUM access serializes
                    # the two engines and made the copies trail the PE.
                    ek_ps = psA.tile([128, CH * 128], F32, tag="ek",
                                     name=f"ek_{j}_{ch}")
                    v_ps = psA.tile([128, CH * 128], F32, tag="vv",
                                    name=f"v_{j}_{ch}")
                    for t in range(CH):
                        for k in range(2):
                            rsl = r_t[:, off + 512 * k + 128 * t:
                                         off + 512 * k + 128 * (t + 1)]
                            nc.tensor.matmul(
                                ek_ps[:, 128 * t:128 * (t + 1)], rsl,
                                wkv_t[j][:, 256 * k:256 * k + 128],
                                start=(k == 0), stop=(k == 1),
                            )
                            nc.tensor.matmul(
                                v_ps[:, 128 * t:128 * (t + 1)], rsl,
                                wkv_t[j][:, 256 * k + 128:256 * (k + 1)],
                                start=(k == 0), stop=(k == 1),
                            )
                    ek_sl = ekt_all[:, ch * CH * 128:(ch + 1) * CH * 128]
                    nc.scalar.activation(ek_sl, ek_ps[:], AF.Exp)
                    nc.vector.tensor_copy(
                        vt_v[:, ch * CH:(ch + 1) * CH, 0:128],
                        v_ps[:].rearrange("p (t c) -> p t c", t=CH),
                    )

                def pass2(j, ch):
                    # ctx accumulation: ek-tile stationary, [v | 1] streamed.
                    # out[c, d] = sum_pix ek[pix, c] v[pix, d]; col 128 = zk.
                    for t in range(ch * CH, (ch + 1) * CH):
                        nc.tensor.matmul(
                            ctx_ps_b[j][:, 0:129],
                            ekt_all[:, 128 * t:128 * (t + 1)],
                            vt_all[:, 129 * t:129 * (t + 1)],
                            start=(t == 0), stop=(t == nt - 1),
                        )

                x_hold = [None]

                def qchunk(i):
                    base = i * 512
                    if i % 2 == 0:
                        pr = i // 2
                        x_hold[0] = iopool.tile([128, 2048], BF16,
                                                tag="xchunk", name=f"x_{pr}")
                        dma_eng = nc.gpsimd if pr % 2 == 0 else nc.sync
                        dma_eng.dma_start(
                            x_hold[0][:], x[128 * pr:128 * (pr + 1), :])
                    x_t = x_hold[0]
                    off = 1024 * (i % 2)
                    q_ps = psQ.tile([128, 512], F32, tag="q", name=f"q_{i}")
                    for k in range(2):
                        nc.tensor.matmul(
                            q_ps[:], wq_t[:, 128 * k:128 * (k + 1)],
                            x_t[:, off + 512 * k:off + 512 * (k + 1)],
                            start=(k == 0), stop=(k == 1),
                        )
                    nc.scalar.activation(
                        expq[:, base:base + 512], q_ps[:], AF.Exp)

                def zqgroup(tc4):
                    # zq = per-head sums of expq (4 col-tiled concurrent MMs),
                    # reciprocal, then matmul-broadcast back over the 32
                    # partitions of each head and normalize expq in place.
                    zq_ps = psQ.tile([128, 512], F32, tag="q", name=f"zq_{tc4}")
                    for u in range(4):
                        t = 4 * tc4 + u
                        nc.tensor.matmul(
                            zq_ps[32 * u:32 * u + 32, :], ones4_t[:],
                            expq[:, 512 * t:512 * (t + 1)],
                            start=True, stop=True,
                            tile_position=(0, 32 * u),
                        )
                    zq_f = wkpool.tile([128, 512], F32, tag="zq_f",
                                       name=f"zqf_{tc4}")
                    nc.vector.reciprocal_approx_fast(zq_f[:], zq_ps[:])
                    nc.vector.tensor_copy(
                        zqr[:, 512 * tc4:512 * (tc4 + 1)], zq_f[:])
                    for t in range(4 * tc4, 4 * tc4 + 4):
                        u = t % 4
                        zqb_ps = psQ.tile([128, 512], F32, tag="q",
                                          name=f"zqb_{t}")
                        nc.tensor.matmul(
                            zqb_ps[:], ones4T_t[32 * u:32 * u + 4, :],
                            zqr[32 * u:32 * u + 4,
                                512 * tc4:512 * (tc4 + 1)],
                            start=True, stop=True,
                            tile_position=(32 * u, 0),
                        )
                        nc.vector.tensor_mul(
                            expq[:, 512 * t:512 * (t + 1)],
                            expq[:, 512 * t:512 * (t + 1)],
                            zqb_ps[:],
                        )

                # WF_j = WoT_j^T-blocks @ ctx_norm_j per head.  Branch 0's
                # half runs in branch 1's shadow — safe now that each
                # branch's ctx lives in its own PSUM bank.  The tiny WF
                # matmul scratch borrows a rotating psQ buffer (it is copied
                # to SBUF immediately, so the bank recycles right away).
                wf0_sb = ppool.tile([128, 256], F32, tag="wf0_sb")

                def wf_part(j):
                    nc.vector.reciprocal_approx_fast(
                        recips[:, j:j + 1], ctx_ps_b[j][:, 128:129])
                    for h in range(4):
                        nc.vector.tensor_scalar_mul(
                            compact[32 * h:32 * (h + 1), 32 * j:32 * j + 32],
                            ctx_ps_b[j][32 * h:32 * (h + 1),
                                        32 * h:32 * (h + 1)],
                            recips[32 * h:32 * (h + 1), j:j + 1],
                        )
                    wfq = psQ.tile([128, 512], F32, tag="q", name=f"wf_{j}")
                    for h in range(4):
                        nc.tensor.matmul(
                            wfq[32 * h:32 * (h + 1), 0:256],
                            compact[32 * h:32 * (h + 1), 32 * j:32 * j + 32],
                            wo_t[32 * h:32 * (h + 1), 256 * j:256 * (j + 1)],
                            start=True, stop=True,
                            tile_position=(32 * h, 32 * h),
                        )
                    if j == 0:
                        nc.vector.tensor_copy(wf0_sb[:], wfq[:, 0:256])
                    else:
                        nc.vector.tensor_add(
                            wft_sb[:], wf0_sb[:], wfq[:, 0:256])

                # ---- branches; q projection spread across both to even out
                # the DMA load (r + x/2 per branch).  Remaining weight DMAs
                # are dripped in between the chunk DMAs. ----
                # qchunks: 8 spread over ch 3..17, 8 packed into ch 18..25,
                # so (i) the first two chunk-pairs of r get the whole DMA
                # ramp to themselves (the early kv stalls were r-pairs
                # arriving late while x pair 0 competed), and (ii) the last
                # zqgroup still finishes ~5 chunks before the branch ends.
                ZQ_AT = {10: 0, 18: 1, 22: 2, 26: 3}
                for j in range(2):
                    for ch in range(nch):
                        pass1(j, ch)
                        if j == 0 and ch == 1:
                            nc.gpsimd.dma_start(wq_t[:], wq[:, :])
                        if j == 0 and ch == 5:
                            nc.sync.dma_start(ones4T_t[:], ones4T[:, :])
                        if j == 0 and ch == 8:
                            nc.sync.dma_start(wkv_t[1][:], wkv2[:, :])
                        if j == 0 and ch == 10:
                            nc.gpsimd.dma_start(wo_t[:], wo[:, :])
                        if ch > 0:
                            pass2(j, ch - 1)
                        if ch % 2 == 1 and 3 <= ch <= 17:
                            qchunk(16 * j + (ch - 3) // 2)
                        elif 18 <= ch <= 25:
                            qchunk(16 * j + 8 + (ch - 18))
                        if ch in ZQ_AT:
                            zqgroup(4 * j + ZQ_AT[ch])
                    pass2(j, nch - 1)
                    wf_part(j)

            # ======= Phase C: y = WF @ expq, streamed over pixel tiles ======
            with (
                tc.tile_pool(name="psC", bufs=4, space="PSUM") as psC,
                tc.tile_pool(name="ysb", bufs=6) as ysbpool,
            ):
                y_hold = [None]
                for t in range(nt512):
                    y_ps = psC.tile([128, 1024], F32, tag="y", name=f"y_{t}")
                    for m in range(2):
                        nc.tensor.matmul(
                            y_ps[:, 512 * m:512 * (m + 1)],
                            wft_sb[:, 128 * m:128 * (m + 1)],
                            expq[:, 512 * t:512 * (t + 1)],
                            start=True, stop=True,
                        )
                    if t % 2 == 0:
                        y_hold[0] = ysbpool.tile([128, 2048], BF16, tag="ysb",
                                                 name=f"ysb_{t // 2}")
                    y_sb = y_hold[0]
                    sl = y_sb[:, 1024 * (t % 2):1024 * (t % 2) + 1024]
                    if t % 2 == 0:
                        nc.vector.tensor_copy(sl, y_ps[:])
                        if t == 0:
                            # kick the output stream one cast earlier: the
                            # out-DMA drain sets phase C's end time
                            nc.sync.dma_start(y[0:128, 0:1024],
                                              y_sb[:, 0:1024])
                    else:
                        nc.scalar.copy(sl, y_ps[:])
                        pr = t // 2
                        if t == 1:
                            nc.gpsimd.dma_start(y[0:128, 1024:2048],
                                                y_sb[:, 1024:2048])
                        else:
                            dma_eng = nc.sync if pr % 2 == 0 else nc.gpsimd
                            dma_eng.dma_start(
                                y[128 * pr:128 * (pr + 1), :], y_sb[:])

    nc.compile()
    return nc


def _consts():
    ones4 = np.zeros((128, 32), dtype=ml_dtypes.bfloat16)
    for col in range(32):
        a = col % 4
        ones4[32 * a:32 * (a + 1), col] = 1
    ones4T = np.zeros((128, 128), dtype=ml_dtypes.bfloat16)
    for u in range(4):
        for a in range(4):
            ones4T[32 * u + a, 32 * a:32 * (a + 1)] = 1
    return ones4, ones4T


def _ktile(wT):
    """[C_in, C_out] -> [128, (C_in//128)*C_out] k-tiles along the free dim."""
    kin = wT.shape[0] // 128
    return np.concatenate([wT[128 * k:128 * (k + 1), :] for k in range(kin)], axis=1)


def _chunkmajor(arr, n_loc=N):
    """[256, n] -> [npair*128, 2048]: pair pr holds pixels 1024pr..1024pr+1024
    at col = 1024*(ch%2) + 512k + c (k = channel k-tile)."""
    npair = n_loc // 1024
    # [k, p, pr, e, c] -> [pr, p, e, k, c]
    a = arr.reshape(2, 128, npair, 2, 512).transpose(2, 1, 3, 0, 4)
    return a.reshape(npair * 128, 2048)


def _unchunk_y(yarr, n_loc=N):
    """[npair*128, 2048] -> [256, n]: per pair pr, col 1024e + 512m + c of
    partition p is y[128m + p, 512*(2pr + e) + c]."""
    npair = n_loc // 1024
    # [pr, p, e, m, c] -> [m, p, pr, e, c]
    a = yarr.reshape(npair, 128, 2, 2, 512).transpose(3, 1, 0, 2, 4)
    return a.reshape(C, n_loc)


def make_in_maps(x, ref_1, ref_2, Wq, Wk1, Wk2, Wv1, Wv2, Wo, n_loc=N):
    bf = ml_dtypes.bfloat16
    ones4, ones4T = _consts()
    xf = np.asarray(x).reshape(B, C, -1)
    r1f = np.asarray(ref_1).reshape(B, C, -1)
    r2f = np.asarray(ref_2).reshape(B, C, -1)
    WqT, WoT = np.asarray(Wq).T, np.asarray(Wo).T
    WkT = [np.asarray(Wk1).T, np.asarray(Wk2).T]
    WvT = [np.asarray(Wv1).T, np.asarray(Wv2).T]
    gw = {}
    for g in range(2):
        sl = slice(128 * g, 128 * (g + 1))
        wq_g = np.ascontiguousarray(_ktile(WqT[:, sl])).astype(bf)
        wkv_g = [np.ascontiguousarray(
            _ktile(np.concatenate([WkT[j][:, sl], WvT[j][:, sl]], axis=1))
        ).astype(bf) for j in range(2)]
        # Wo rows for our concat channels: branch j block at cols 256j
        wo_g = np.ascontiguousarray(np.concatenate(
            [WoT[128 * g:128 * (g + 1), :],
             WoT[256 + 128 * g:256 + 128 * (g + 1), :]],
            axis=1)).astype(bf)
        gw[g] = (wq_g, wkv_g[0], wkv_g[1], wo_g)
    in_maps = []
    for core in range(N_CORES):
        b, g = core // 2, core % 2
        wq_g, wkv1_g, wkv2_g, wo_g = gw[g]
        in_maps.append({
            "x": np.ascontiguousarray(_chunkmajor(xf[b, :, :n_loc], n_loc)).astype(bf),
            "r1": np.ascontiguousarray(_chunkmajor(r1f[b, :, :n_loc], n_loc)).astype(bf),
            "r2": np.ascontiguousarray(_chunkmajor(r2f[b, :, :n_loc], n_loc)).astype(bf),
            "wq": wq_g, "wkv1": wkv1_g, "wkv2": wkv2_g, "wo": wo_g,
            "ones4": ones4, "ones4T": ones4T,
        })
    return in_maps


_NC_CACHE = {}


def kernel(x, ref_1, ref_2, Wq, Wk1, Wk2, Wv1, Wv2, Wo, _trace=False):
    n_loc = N
    if n_loc not in _NC_CACHE:
        _NC_CACHE[n_loc] = build_nc(n_loc)
    nc = _NC_CACHE[n_loc]
    in_maps = make_in_maps(x, ref_1, ref_2, Wq, Wk1, Wk2, Wv1, Wv2, Wo, n_loc)
    res = run_bass_kernel_spmd(nc, in_maps, core_ids=list(range(N_CORES)),
                               trace=_trace)
    out = np.empty((B, C, n_loc), dtype=np.float32)
    for b in range(B):
        out[b] = (_unchunk_y(res.results[2 * b]["y"].astype(np.float32), n_loc)
                  + _unchunk_y(res.results[2 * b + 1]["y"].astype(np.float32), n_loc))
    if _trace:
        kernel.last_results = res
    return out.reshape(B, C, H, W)
